# revision 1
# baseline (speedup 1.0000x reference)
"""Trainium2 Bass kernel for nn_LSTMSimple: 2-layer LSTM + BatchNorm + dense head.

Strategy: data-parallel over batch (128 -> 16 per core, 8 cores).
Per core:
  A) Z1 = X @ Wi1 + b1 precomputed for all timesteps (PE, big matmuls) -> HBM
  B) L1 recurrence: per step z = Z1[t] + h @ Wh1 (Z1[t] injected into the PSUM
     accumulation group via an identity-stationary matmul), sigmoid/tanh on
     ScalarE, c/h updates on VectorE, h -> h^T via 4 PE transpose matmuls.
     h^T also streamed to HBM (it is the stationary operand of the Z2 matmul).
  C) BN1 batch stats via ScalarE accum_out + one 4KB AllReduce; BN1 is folded
     into the Z2 precompute (scale rows of Wi2, add a bias row) - the
     normalized activations are never materialized.
  E) Z2 = H1bn @ Wi2 + b2 precompute from stored h^T tiles.
  F) L2 recurrence (identical, no state store; keeps final h^T).
  G) BN2 stats AllReduce, folded into Wd1; dense head on PE; out = [1, 16].
Host reorders gate columns from (i,f,g,o) to (i,f,o,g) so one sigmoid op
covers columns 0:1536 and one tanh op covers 1536:2048.
"""

import sys

if '/opt/trn_rl_repo' not in sys.path:
    sys.path.insert(0, '/opt/trn_rl_repo')

import numpy as np

# ---- problem constants (hardcoded per contract) ----
B = 128
T = int(__import__('os').environ.get('LSTM_T', '512'))  # debug knob; harness uses 512
F = 512
H = 512
G4 = 4 * H           # 2048
NCORES = 8
BL = B // NCORES     # 16 batch rows per core
SPC = 8              # timesteps per Z chunk (128 = 8*16 partition rows)
EPS = 1e-5

FP32 = None  # filled after mybir import


def _build_program(t_steps: int):
    import concourse.bacc as bacc
    import concourse.mybir as mybir
    import concourse.tile as tile

    f32 = mybir.dt.float32
    bf16 = mybir.dt.bfloat16
    AF = mybir.ActivationFunctionType

    NCH = t_steps // SPC  # z chunks per layer

    nc = bacc.Bacc("TRN2", target_bir_lowering=False, debug=False,
                   num_devices=NCORES)

    # ---- kernel I/O ----
    d_xT = nc.dram_tensor("xT", [F, t_steps * BL], f32, kind="ExternalInput")
    d_wi1 = nc.dram_tensor("wi1", [F, G4], f32, kind="ExternalInput")
    d_wh1 = nc.dram_tensor("wh1", [H, G4], f32, kind="ExternalInput")
    d_b1 = nc.dram_tensor("b1row", [1, G4], f32, kind="ExternalInput")
    d_wi2 = nc.dram_tensor("wi2", [H, G4], f32, kind="ExternalInput")
    d_wh2 = nc.dram_tensor("wh2", [H, G4], f32, kind="ExternalInput")
    d_b2 = nc.dram_tensor("b2row", [1, G4], f32, kind="ExternalInput")
    d_bn1s = nc.dram_tensor("bn1s", [128, 4], f32, kind="ExternalInput")
    d_bn1b = nc.dram_tensor("bn1b", [128, 4], f32, kind="ExternalInput")
    d_bn2s = nc.dram_tensor("bn2s", [128, 4], f32, kind="ExternalInput")
    d_bn2b = nc.dram_tensor("bn2b", [128, 4], f32, kind="ExternalInput")
    d_wd1 = nc.dram_tensor("wd1", [H, 16], f32, kind="ExternalInput")
    d_bd1 = nc.dram_tensor("bd1c", [16, 1], f32, kind="ExternalInput")
    d_wd2 = nc.dram_tensor("wd2", [16, 1], f32, kind="ExternalInput")
    d_bd2 = nc.dram_tensor("bd2c", [1, 1], f32, kind="ExternalInput")
    d_ia = nc.dram_tensor("IA", [128, 16], f32, kind="ExternalInput")
    d_ib = nc.dram_tensor("IB", [128, 16], f32, kind="ExternalInput")
    d_i16 = nc.dram_tensor("I16", [16, 16], f32, kind="ExternalInput")
    d_ones = nc.dram_tensor("ones1", [1, 128], f32, kind="ExternalInput")
    d_out = nc.dram_tensor("out", [1, 16], f32, kind="ExternalOutput")

    with tile.TileContext(nc) as tc:
        with (
            tc.tile_pool(name="const", bufs=1) as cpool,
            tc.tile_pool(name="wpool", bufs=1) as wpool,
            tc.tile_pool(name="zstr", bufs=3) as zpool,
            tc.tile_pool(name="xt", bufs=8) as xpool,
            tc.tile_pool(name="gat", bufs=2) as gpool,
            tc.tile_pool(name="tmp", bufs=2) as tpool,
            tc.tile_pool(name="ps", bufs=2, space="PSUM") as pspool,
            tc.tile_pool(name="dram", bufs=1, space="DRAM") as dpool,
        ):
            # ---- constants / weights in SBUF ----
            ia_sb = cpool.tile([128, 16], f32, tag="ia")
            ib_sb = cpool.tile([128, 16], f32, tag="ib")
            i16_sb = cpool.tile([16, 16], f32, tag="i16")
            ones_sb = cpool.tile([1, 128], f32, tag="ones")
            nc.sync.dma_start(ia_sb[:, :], d_ia[:, :])
            nc.sync.dma_start(ib_sb[:, :], d_ib[:, :])
            nc.sync.dma_start(i16_sb[:, :], d_i16[:, :])
            nc.sync.dma_start(ones_sb[:, :], d_ones[:, :])

            wi_sb = wpool.tile([128, 4, G4], f32, tag="wi")  # Wi1, later Wi2
            for kc in range(4):
                nc.sync.dma_start(wi_sb[:, kc, :], d_wi1[kc * 128:(kc + 1) * 128, :])
            b1_sb = cpool.tile([1, G4], f32, tag="brow0")
            nc.sync.dma_start(b1_sb[:, :], d_b1[:, :])

            wh_sb = wpool.tile([128, 4, G4], f32, tag="wh")  # Wh1, later Wh2
            for kc in range(4):
                nc.sync.dma_start(wh_sb[:, kc, :], d_wh1[kc * 128:(kc + 1) * 128, :])

            # ---- DRAM intermediates ----
            z1d = dpool.tile([NCH, 128, G4], f32, tag="z1d")
            z2d = dpool.tile([NCH, 128, G4], f32, tag="z2d")
            h1t = dpool.tile([128, 4, t_steps, 16], f32, tag="h1t")
            cc1_in = dpool.tile([128, 8], f32, tag="cc1i")
            cc1_out = dpool.tile([128, 8], f32, tag="cc1o")
            cc2_in = dpool.tile([128, 8], f32, tag="cc2i")
            cc2_out = dpool.tile([128, 8], f32, tag="cc2o")

            # ================= Phase A: Z1 precompute =================
            def z_precompute(zd, get_lhs_tile, rhs_w, bias_row):
                """zd[c] = lhsT_c.T @ W + bias_row for all row chunks."""
                for c in range(NCH):
                    lhs = [get_lhs_tile(c, kc) for kc in range(4)]
                    zp = pspool.tile([128, G4], f32, tag="ps")
                    for kc in range(4):
                        for nb in range(4):
                            nc.tensor.matmul(
                                zp[:, nb * 512:(nb + 1) * 512],
                                lhs[kc][:, :],
                                rhs_w[:, kc, nb * 512:(nb + 1) * 512],
                                start=(kc == 0), stop=False)
                    for nb in range(4):
                        nc.tensor.matmul(
                            zp[:, nb * 512:(nb + 1) * 512],
                            ones_sb[:, :],
                            bias_row[:, nb * 512:(nb + 1) * 512],
                            start=False, stop=True)
                    zsb = zpool.tile([128, G4], f32, tag="zstr")
                    nc.vector.tensor_copy(zsb[:, :], zp[:, :])
                    nc.sync.dma_start(zd[c], zsb[:, :])

            def get_x_tile(c, kc):
                xt = xpool.tile([128, 128], f32, tag="xt")
                nc.sync.dma_start(
                    xt[:, :], d_xT[kc * 128:(kc + 1) * 128, c * 128:(c + 1) * 128])
                return xt

            z_precompute(z1d, get_x_tile, wi_sb, b1_sb)

            # ---- persistent recurrence state ----
            hT_sb = cpool.tile([128, 4, 16], f32, tag="hT")
            c_sb = cpool.tile([16, 512], f32, tag="cst")

            # ================= recurrence =================
            def lstm_layer(zd, wh, store_h1t):
                nc.vector.memset(hT_sb[:, :, :], 0.0)
                nc.vector.memset(c_sb[:, :], 0.0)
                zch = {0: zpool.tile([128, G4], f32, tag="zstr", name="zch0")}
                nc.sync.dma_start(zch[0][:, :], zd[0])
                for t in range(t_steps):
                    cix, ts = divmod(t, SPC)
                    if ts == 0 and cix + 1 < NCH:
                        zch[cix + 1] = zpool.tile([128, G4], f32, tag="zstr", name="zch")
                        nc.sync.dma_start(zch[cix + 1][:, :], zd[cix + 1])
                    if cix - 2 in zch:
                        del zch[cix - 2]
                    base = 32 * (ts // 2)
                    sel = ia_sb if ts % 2 == 0 else ib_sb
                    zp = pspool.tile([16, G4], f32, tag="ps")
                    for nb in range(4):
                        nc.tensor.matmul(
                            zp[:, nb * 512:(nb + 1) * 512],
                            sel[base:base + 32, :],
                            zch[cix][base:base + 32, nb * 512:(nb + 1) * 512],
                            start=True, stop=False, tile_position=(base, 0))
                    for kc in range(4):
                        for nb in range(4):
                            nc.tensor.matmul(
                                zp[:, nb * 512:(nb + 1) * 512],
                                hT_sb[:, kc, :],
                                wh[:, kc, nb * 512:(nb + 1) * 512],
                                start=False, stop=(kc == 3))
                    gates = gpool.tile([16, G4], f32, tag="gates")
                    nc.scalar.activation(gates[:, 0:1536], zp[:, 0:1536], AF.Sigmoid)
                    nc.scalar.activation(gates[:, 1536:G4], zp[:, 1536:G4], AF.Tanh)
                    t1 = tpool.tile([16, 512], f32, tag="t1")
                    t2 = tpool.tile([16, 512], f32, tag="t2")
                    nc.vector.tensor_mul(t1[:, :], gates[:, 0:512], gates[:, 1536:G4])
                    nc.vector.tensor_mul(t2[:, :], gates[:, 512:1024], c_sb[:, :])
                    nc.vector.tensor_add(c_sb[:, :], t1[:, :], t2[:, :])
                    tcs = tpool.tile([16, 512], f32, tag="tc")
                    nc.scalar.activation(tcs[:, :], c_sb[:, :], AF.Tanh)
                    hs = tpool.tile([16, 512], f32, tag="h")
                    nc.vector.tensor_mul(hs[:, :], gates[:, 1024:1536], tcs[:, :])
                    htp = pspool.tile([128, 4, 16], f32, tag="ps")
                    for kc in range(4):
                        nc.tensor.matmul(
                            htp[:, kc, :], hs[:, kc * 128:(kc + 1) * 128],
                            i16_sb[:, :], start=(kc == 0), stop=(kc == 3),
                            is_transpose=True)
                    nc.vector.tensor_copy(hT_sb[:, :, :], htp[:, :, :])
                    if store_h1t:
                        nc.sync.dma_start(h1t[:, :, t, :], hT_sb[:, :, :])

            lstm_layer(z1d, wh_sb, store_h1t=True)

            # ================= Phase C: BN1 stats =================
            psum_parts = cpool.tile([128, 4, 4], f32, tag="p_sum")
            psq_parts = cpool.tile([128, 4, 4], f32, tag="p_sq")
            TCH = t_steps // 4  # stat chunk in timesteps
            for kc in range(4):
                for qi in range(4):
                    hb = zpool.tile([128, TCH, 16], f32, tag="zstr")
                    nc.sync.dma_start(
                        hb[:, :, :], h1t[:, kc, qi * TCH:(qi + 1) * TCH, :])
                    tr1 = tpool.tile([128, TCH, 16], bf16, tag="trash")
                    nc.scalar.activation(tr1[:, :, :], hb[:, :, :], AF.Identity,
                                         accum_out=psum_parts[:, kc, qi:qi + 1])
                    tr2 = tpool.tile([128, TCH, 16], bf16, tag="trash")
                    nc.scalar.activation(tr2[:, :, :], hb[:, :, :], AF.Square,
                                         accum_out=psq_parts[:, kc, qi:qi + 1])
            allred = cpool.tile([128, 8], f32, tag="allred")
            nc.vector.tensor_reduce(allred[:, 0:4], psum_parts[:, :, :],
                                    mybir.AxisListType.X, mybir.AluOpType.add)
            nc.vector.tensor_reduce(allred[:, 4:8], psq_parts[:, :, :],
                                    mybir.AxisListType.X, mybir.AluOpType.add)
            nc.sync.dma_start(cc1_in[:, :], allred[:, :])
            nc.gpsimd.collective_compute(
                "AllReduce", mybir.AluOpType.add,
                replica_groups=[list(range(NCORES))],
                ins=[cc1_in.opt()], outs=[cc1_out.opt()])
            nc.sync.dma_start(allred[:, :], cc1_out[:, :])

            bn1s_sb = cpool.tile([128, 4], f32, tag="bn1s")
            bn1b_sb = cpool.tile([128, 4], f32, tag="bn1b")
            nc.sync.dma_start(bn1s_sb[:, :], d_bn1s[:, :])
            nc.sync.dma_start(bn1b_sb[:, :], d_bn1b[:, :])

            def bn_fold(allred_sb, n_count, bns, bnb):
                """Return (a, d): bn(x) = x*a + d per feature, [128,4] tiles."""
                mu = cpool.tile([128, 4], f32, tag=f"mu{n_count}")
                ex2 = cpool.tile([128, 4], f32, tag=f"ex2{n_count}")
                nc.vector.tensor_scalar_mul(mu[:, :], allred_sb[:, 0:4], 1.0 / n_count)
                nc.vector.tensor_scalar_mul(ex2[:, :], allred_sb[:, 4:8], 1.0 / n_count)
                var = cpool.tile([128, 4], f32, tag=f"var{n_count}")
                nc.vector.tensor_mul(var[:, :], mu[:, :], mu[:, :])
                nc.vector.tensor_sub(var[:, :], ex2[:, :], var[:, :])
                nc.vector.tensor_scalar_add(var[:, :], var[:, :], EPS)
                sd = cpool.tile([128, 4], f32, tag=f"sd{n_count}")
                nc.scalar.activation(sd[:, :], var[:, :], AF.Sqrt)
                r0 = cpool.tile([128, 4], f32, tag=f"r0{n_count}")
                nc.vector.reciprocal(r0[:, :], sd[:, :])
                # one Newton step: r1 = r0 * (1.5 - 0.5 * var * r0^2)
                e1 = cpool.tile([128, 4], f32, tag=f"e1{n_count}")
                nc.vector.tensor_mul(e1[:, :], r0[:, :], r0[:, :])
                nc.vector.tensor_mul(e1[:, :], e1[:, :], var[:, :])
                nc.vector.tensor_scalar(e1[:, :], e1[:, :], -0.5, 1.5,
                                        mybir.AluOpType.mult, mybir.AluOpType.add)
                nc.vector.tensor_mul(r0[:, :], r0[:, :], e1[:, :])
                a = cpool.tile([128, 4], f32, tag=f"a{n_count}")
                dv = cpool.tile([128, 4], f32, tag=f"d{n_count}")
                nc.vector.tensor_mul(a[:, :], r0[:, :], bns[:, :])
                nc.vector.tensor_mul(dv[:, :], mu[:, :], a[:, :])
                nc.vector.tensor_sub(dv[:, :], bnb[:, :], dv[:, :])
                return a, dv

            a1, d1v = bn_fold(allred, B * t_steps, bn1s_sb, bn1b_sb)

            # ================= Phase D: fold BN1 into Wi2 =================
            for kc in range(4):
                nc.sync.dma_start(wi_sb[:, kc, :], d_wi2[kc * 128:(kc + 1) * 128, :])
            b2_sb = cpool.tile([1, G4], f32, tag="brow1")
            nc.sync.dma_start(b2_sb[:, :], d_b2[:, :])

            r2_ps = pspool.tile([1, G4], f32, tag="ps")
            for kc in range(4):
                for nb in range(4):
                    nc.tensor.matmul(r2_ps[:, nb * 512:(nb + 1) * 512],
                                     d1v[:, kc:kc + 1],
                                     wi_sb[:, kc, nb * 512:(nb + 1) * 512],
                                     start=(kc == 0), stop=False)
            for nb in range(4):
                nc.tensor.matmul(r2_ps[:, nb * 512:(nb + 1) * 512],
                                 ones_sb[:, 0:1], b2_sb[:, nb * 512:(nb + 1) * 512],
                                 start=False, stop=True)
            r2_sb = cpool.tile([1, G4], f32, tag="brow0")  # reuse b1row slot
            nc.vector.tensor_copy(r2_sb[:, :], r2_ps[:, :])
            for kc in range(4):
                nc.vector.tensor_scalar_mul(wi_sb[:, kc, :], wi_sb[:, kc, :],
                                            a1[:, kc:kc + 1])

            # ================= Phase E: Z2 precompute =================
            def get_h1t_tile(c, kc):
                ht = xpool.tile([128, SPC, 16], f32, tag="xt")
                nc.sync.dma_start(ht[:, :, :],
                                  h1t[:, kc, c * SPC:(c + 1) * SPC, :])
                return ht

            z_precompute(z2d, get_h1t_tile, wi_sb, r2_sb)

            # ================= Phase F: L2 recurrence =================
            for kc in range(4):
                nc.sync.dma_start(wh_sb[:, kc, :], d_wh2[kc * 128:(kc + 1) * 128, :])
            lstm_layer(z2d, wh_sb, store_h1t=False)

            # ================= Phase G: BN2 + dense head =================
            s2 = cpool.tile([128, 4], f32, tag="s2")
            q2 = cpool.tile([128, 4], f32, tag="q2")
            tr3 = cpool.tile([128, 4, 16], bf16, tag="tr3")
            for kc in range(4):
                nc.scalar.activation(tr3[:, kc, :], hT_sb[:, kc, :], AF.Identity,
                                     accum_out=s2[:, kc:kc + 1])
                nc.scalar.activation(tr3[:, kc, :], hT_sb[:, kc, :], AF.Square,
                                     accum_out=q2[:, kc:kc + 1])
            allred2 = cpool.tile([128, 8], f32, tag="allred2")
            nc.vector.tensor_copy(allred2[:, 0:4], s2[:, :])
            nc.vector.tensor_copy(allred2[:, 4:8], q2[:, :])
            nc.sync.dma_start(cc2_in[:, :], allred2[:, :])
            nc.gpsimd.collective_compute(
                "AllReduce", mybir.AluOpType.add,
                replica_groups=[list(range(NCORES))],
                ins=[cc2_in.opt()], outs=[cc2_out.opt()])
            nc.sync.dma_start(allred2[:, :], cc2_out[:, :])

            bn2s_sb = cpool.tile([128, 4], f32, tag="bn2s")
            bn2b_sb = cpool.tile([128, 4], f32, tag="bn2b")
            nc.sync.dma_start(bn2s_sb[:, :], d_bn2s[:, :])
            nc.sync.dma_start(bn2b_sb[:, :], d_bn2b[:, :])
            a2, d2v = bn_fold(allred2, B, bn2s_sb, bn2b_sb)

            wd1_sb = cpool.tile([128, 4, 16], f32, tag="wd1")
            for kc in range(4):
                nc.sync.dma_start(wd1_sb[:, kc, :], d_wd1[kc * 128:(kc + 1) * 128, :])
            bd1_sb = cpool.tile([16, 1], f32, tag="bd1")
            nc.sync.dma_start(bd1_sb[:, :], d_bd1[:, :])
            wd2_sb = cpool.tile([16, 1], f32, tag="wd2")
            nc.sync.dma_start(wd2_sb[:, :], d_wd2[:, :])
            bd2_sb = cpool.tile([1, 1], f32, tag="bd2")
            nc.sync.dma_start(bd2_sb[:, :], d_bd2[:, :])

            # bias_d1[j] = sum_h Wd1[h, j] * d2v[h] + bd1[j]  (psum [16, 1])
            bd1_ps = pspool.tile([16, 1], f32, tag="ps")
            for kc in range(4):
                nc.tensor.matmul(bd1_ps[:, :], wd1_sb[:, kc, :], d2v[:, kc:kc + 1],
                                 start=(kc == 0), stop=(kc == 3))
            biasd1 = cpool.tile([16, 1], f32, tag="biasd1")
            nc.vector.tensor_copy(biasd1[:, :], bd1_ps[:, :])
            nc.vector.tensor_add(biasd1[:, :], biasd1[:, :], bd1_sb[:, :])
            # scale Wd1 rows by a2 (after the bias matmuls read the raw Wd1)
            for kc in range(4):
                nc.vector.tensor_scalar_mul(wd1_sb[:, kc, :], wd1_sb[:, kc, :],
                                            a2[:, kc:kc + 1])
            # d1T[j, b] = tanh( sum_h Wd1'[h,j] * hT[h,b] + bias_d1[j] )
            d1_ps = pspool.tile([16, 16], f32, tag="ps")
            for kc in range(4):
                nc.tensor.matmul(d1_ps[:, :], wd1_sb[:, kc, :], hT_sb[:, kc, :],
                                 start=(kc == 0), stop=(kc == 3))
            d1T = cpool.tile([16, 16], f32, tag="d1T")
            nc.scalar.activation(d1T[:, :], d1_ps[:, :], AF.Tanh, bias=biasd1[:, 0:1])
            # out[0, b] = sum_j Wd2[j] * d1T[j, b] + bd2
            o_ps = pspool.tile([1, 16], f32, tag="ps")
            nc.tensor.matmul(o_ps[:, :], wd2_sb[:, :], d1T[:, :],
                             start=True, stop=True)
            out_sb = cpool.tile([1, 16], f32, tag="outsb")
            nc.scalar.activation(out_sb[:, :], o_ps[:, :], AF.Identity,
                                 bias=bd2_sb[:, 0:1])
            nc.sync.dma_start(d_out[:, :], out_sb[:, :])

    nc.compile()
    return nc


_PROG_CACHE = {}


def _get_program(t_steps):
    if t_steps not in _PROG_CACHE:
        _PROG_CACHE[t_steps] = _build_program(t_steps)
    return _PROG_CACHE[t_steps]


def kernel(x, Wi1, Wh1, b1, Wi2, Wh2, b2, bn1_scale, bn1_bias,
           bn2_scale, bn2_bias, Wd1, bd1, Wd2, bd2):
    from concourse.bass_utils import run_bass_kernel_spmd

    x = np.asarray(x, dtype=np.float32)
    t_steps = x.shape[1]
    nc = _get_program(t_steps)

    # gate reorder (i,f,g,o) -> (i,f,o,g)
    perm = np.concatenate([np.arange(0, 512), np.arange(512, 1024),
                           np.arange(1536, 2048), np.arange(1024, 1536)])
    wi1 = np.ascontiguousarray(np.asarray(Wi1, np.float32)[:, perm])
    wh1 = np.ascontiguousarray(np.asarray(Wh1, np.float32)[:, perm])
    b1p = np.asarray(b1, np.float32)[perm].reshape(1, G4)
    wi2 = np.ascontiguousarray(np.asarray(Wi2, np.float32)[:, perm])
    wh2 = np.ascontiguousarray(np.asarray(Wh2, np.float32)[:, perm])
    b2p = np.asarray(b2, np.float32)[perm].reshape(1, G4)

    def col4(v):
        return np.ascontiguousarray(np.asarray(v, np.float32).reshape(4, 128).T)

    ia = np.zeros((128, 16), np.float32)
    ib = np.zeros((128, 16), np.float32)
    for g in range(4):
        for j in range(16):
            ia[32 * g + j, j] = 1.0
            ib[32 * g + 16 + j, j] = 1.0
    common = {
        "wi1": wi1, "wh1": wh1, "b1row": b1p,
        "wi2": wi2, "wh2": wh2, "b2row": b2p,
        "bn1s": col4(bn1_scale), "bn1b": col4(bn1_bias),
        "bn2s": col4(bn2_scale), "bn2b": col4(bn2_bias),
        "wd1": np.asarray(Wd1, np.float32),
        "bd1c": np.asarray(bd1, np.float32).reshape(16, 1),
        "wd2": np.asarray(Wd2, np.float32).reshape(16, 1),
        "bd2c": np.asarray(bd2, np.float32).reshape(1, 1),
        "IA": ia, "IB": ib, "I16": np.eye(16, dtype=np.float32),
        "ones1": np.ones((1, 128), np.float32),
    }
    in_maps = []
    for ci in range(NCORES):
        xs = x[ci * BL:(ci + 1) * BL]                    # [16, T, F]
        xT = np.ascontiguousarray(xs.transpose(2, 1, 0).reshape(F, t_steps * BL))
        m = dict(common)
        m["xT"] = xT
        in_maps.append(m)

    global _LAST_IN_MAPS
    _LAST_IN_MAPS = in_maps
    res = run_bass_kernel_spmd(nc, in_maps, core_ids=list(range(NCORES)))
    y = np.concatenate(
        [res.results[ci]["out"].reshape(16, 1) for ci in range(NCORES)], axis=0)
    return y.astype(np.float32)



# revision 24
# speedup vs baseline: 392.4658x; 392.4658x over previous
"""Trainium2 Bass kernel for nn_LSTMSimple: 2-layer LSTM + BatchNorm + dense head.

Strategy: data-parallel over batch (128 -> 16 per core, 8 cores).
Per core:
  A) Z1 = X @ Wi1 + b1 precomputed for all timesteps (PE, big matmuls) -> HBM
  B) L1 recurrence: per step z = Z1[t] + h @ Wh1 (Z1[t] injected into the PSUM
     accumulation group via an identity-stationary matmul), sigmoid/tanh on
     ScalarE, c/h updates on VectorE, h -> h^T via 4 PE transpose matmuls.
     h^T also streamed to HBM (it is the stationary operand of the Z2 matmul).
  C) BN1 batch stats via ScalarE accum_out + one 4KB AllReduce; BN1 is folded
     into the Z2 precompute (scale rows of Wi2, add a bias row) - the
     normalized activations are never materialized.
  E) Z2 = H1bn @ Wi2 + b2 precompute from stored h^T tiles.
  F) L2 recurrence (identical, no state store; keeps final h^T).
  G) BN2 stats AllReduce, folded into Wd1; dense head on PE; out = [1, 16].
Host reorders gate columns from (i,f,g,o) to (i,f,o,g) so one sigmoid op
covers columns 0:1536 and one tanh op covers 1536:2048.

v5 over the baseline: large matmuls run at 1 cycle/row instead of 4 --
weights/x/h^T tensors are declared native float32r end-to-end (DMA-produced
or TensorCopy-produced, per the BIR verifier's rounding rule), and the Z
stream (x@Wi+b) is stored/injected in bf16. Structure, PSUM layout and
engine choreography are identical to the baseline (all constructs
hardware-proven); only dtypes and the Z staging changed.
"""

import sys

if '/opt/trn_rl_repo' not in sys.path:
    sys.path.insert(0, '/opt/trn_rl_repo')

import numpy as np

# ---- problem constants (hardcoded per contract) ----
B = 128
T = int(__import__('os').environ.get('LSTM_T', '512'))  # debug knob
F = 512
H = 512
G4 = 4 * H           # 2048
NCORES = 8
BL = B // NCORES     # 16 batch rows per core
SPC = 8              # timesteps per Z chunk (128 = 8*16 partition rows)
EPS = 1e-5


def _build_program(t_steps: int):
    import concourse.bacc as bacc
    import concourse.mybir as mybir
    import concourse.tile as tile

    f32 = mybir.dt.float32
    f32r = mybir.dt.float32r
    bf16 = mybir.dt.bfloat16
    AF = mybir.ActivationFunctionType

    NCH = t_steps // SPC  # z chunks per layer

    nc = bacc.Bacc("TRN2", target_bir_lowering=False, debug=False,
                   num_devices=NCORES)

    # ---- kernel I/O ----
    d_xT = nc.dram_tensor("xT", [F, t_steps * BL], f32r, kind="ExternalInput")
    d_wi1 = nc.dram_tensor("wi1", [F, G4], f32r, kind="ExternalInput")
    d_wh1 = nc.dram_tensor("wh1", [H, G4], f32r, kind="ExternalInput")
    d_b1 = nc.dram_tensor("b1row", [1, G4], f32r, kind="ExternalInput")
    d_wi2 = nc.dram_tensor("wi2", [H, G4], f32r, kind="ExternalInput")
    d_wh2 = nc.dram_tensor("wh2", [H, G4], f32r, kind="ExternalInput")
    d_b2 = nc.dram_tensor("b2row", [1, G4], f32r, kind="ExternalInput")
    d_bn1s = nc.dram_tensor("bn1s", [128, 4], f32, kind="ExternalInput")
    d_bn1b = nc.dram_tensor("bn1b", [128, 4], f32, kind="ExternalInput")
    d_bn2s = nc.dram_tensor("bn2s", [128, 4], f32, kind="ExternalInput")
    d_bn2b = nc.dram_tensor("bn2b", [128, 4], f32, kind="ExternalInput")
    d_wd1 = nc.dram_tensor("wd1", [H, 16], f32, kind="ExternalInput")
    d_bd1 = nc.dram_tensor("bd1c", [16, 1], f32, kind="ExternalInput")
    d_wd2 = nc.dram_tensor("wd2", [16, 1], f32, kind="ExternalInput")
    d_bd2 = nc.dram_tensor("bd2c", [1, 1], f32, kind="ExternalInput")
    d_ia = nc.dram_tensor("IA", [128, 16], f32, kind="ExternalInput")
    d_ib = nc.dram_tensor("IB", [128, 16], f32, kind="ExternalInput")
    d_i16 = nc.dram_tensor("I16", [16, 16], f32, kind="ExternalInput")
    d_ones = nc.dram_tensor("ones1", [1, 128], f32r, kind="ExternalInput")
    d_out = nc.dram_tensor("out", [1, 16], f32, kind="ExternalOutput")

    with tile.TileContext(nc) as tc:
        with (
            tc.tile_pool(name="const", bufs=1) as cpool,
            tc.tile_pool(name="wpool", bufs=1) as wpool,
            tc.tile_pool(name="zstr", bufs=3) as zpool,
            tc.tile_pool(name="xt", bufs=8) as xpool,
            tc.tile_pool(name="gat", bufs=2) as gpool,
            tc.tile_pool(name="tmp", bufs=2) as tpool,
            tc.tile_pool(name="ps", bufs=2, space="PSUM") as pspool,
            tc.tile_pool(name="dram", bufs=1, space="DRAM") as dpool,
        ):
            # ---- constants / weights in SBUF ----
            ia_sb = cpool.tile([128, 16], f32, tag="ia")
            ib_sb = cpool.tile([128, 16], f32, tag="ib")
            i16_sb = cpool.tile([16, 16], f32, tag="i16")
            ones_sb = cpool.tile([1, 128], f32r, tag="ones")
            nc.sync.dma_start(ia_sb[:, :], d_ia[:, :])
            nc.sync.dma_start(ib_sb[:, :], d_ib[:, :])
            nc.sync.dma_start(i16_sb[:, :], d_i16[:, :])
            nc.sync.dma_start(ones_sb[:, :], d_ones[:, :])
            ia_bf = cpool.tile([128, 16], bf16, tag="iabf")
            ib_bf = cpool.tile([128, 16], bf16, tag="ibbf")
            nc.vector.tensor_copy(ia_bf[:, :], ia_sb[:, :])
            nc.vector.tensor_copy(ib_bf[:, :], ib_sb[:, :])

            wi_sb = wpool.tile([128, 4, G4], f32r, tag="wi")  # Wi1, later Wi2
            for kc in range(4):
                nc.sync.dma_start(wi_sb[:, kc, :], d_wi1[kc * 128:(kc + 1) * 128, :])
            b1_sb = cpool.tile([1, G4], f32r, tag="brow0")
            nc.sync.dma_start(b1_sb[:, :], d_b1[:, :])

            wh_sb = wpool.tile([128, 4, G4], f32r, tag="wh")  # Wh1, later Wh2
            for kc in range(4):
                nc.sync.dma_start(wh_sb[:, kc, :], d_wh1[kc * 128:(kc + 1) * 128, :])

            # ---- DRAM intermediates ----
            z1d = dpool.tile([NCH, 128, G4], bf16, tag="z1d")
            z2d = dpool.tile([NCH, 128, G4], bf16, tag="z2d")
            h1t = dpool.tile([128, 4, t_steps, 16], f32r, tag="h1t")
            cc1_in = dpool.tile([128, 8], f32, tag="cc1i")
            cc1_out = dpool.tile([128, 8], f32, tag="cc1o")
            cc2_in = dpool.tile([128, 8], f32, tag="cc2i")
            cc2_out = dpool.tile([128, 8], f32, tag="cc2o")

            # ================= Phase A: Z1 precompute =================
            def z_precompute(zd, get_lhs_tile, rhs_w, bias_row):
                """zd[c] = lhsT_c.T @ W + bias_row for all row chunks (bf16)."""
                for c in range(NCH):
                    lhs = [get_lhs_tile(c, kc) for kc in range(4)]
                    zp = pspool.tile([128, G4], f32, tag="ps", name="zp")
                    for kc in range(4):
                        for nb in range(4):
                            nc.tensor.matmul(
                                zp[:, nb * 512:(nb + 1) * 512],
                                lhs[kc][:, :],
                                rhs_w[:, kc, nb * 512:(nb + 1) * 512],
                                start=(kc == 0), stop=False)
                    for nb in range(4):
                        nc.tensor.matmul(
                            zp[:, nb * 512:(nb + 1) * 512],
                            ones_sb[:, :],
                            bias_row[:, nb * 512:(nb + 1) * 512],
                            start=False, stop=True)
                    zsb = zpool.tile([128, G4], bf16, tag="zstr", name="zsb")
                    nc.vector.tensor_copy(zsb[:, :], zp[:, :])
                    nc.sync.dma_start(zd[c], zsb[:, :])

            def get_x_tile(c, kc):
                xt = xpool.tile([128, 128], f32r, tag="xt", name="xt")
                nc.sync.dma_start(
                    xt[:, :], d_xT[kc * 128:(kc + 1) * 128, c * 128:(c + 1) * 128])
                return xt

            z_precompute(z1d, get_x_tile, wi_sb, b1_sb)

            # ---- persistent recurrence state ----
            hT_sb = cpool.tile([128, 4, 16], f32r, tag="hT")
            c_sb = cpool.tile([16, 512], f32, tag="cst")

            # ================= recurrence =================
            def lstm_layer(zd, wh, store_h1t):
                nc.vector.memset(c_sb[:, :], 0.0)
                zch = {0: zpool.tile([128, G4], bf16, tag="zstr", name="zch0")}
                nc.sync.dma_start(zch[0][:, :], zd[0])
                for t in range(t_steps):
                    st0 = (t == 0)  # h == 0: skip the Wh matmuls
                    cix, ts = divmod(t, SPC)
                    if ts == 0 and cix + 1 < NCH:
                        zch[cix + 1] = zpool.tile([128, G4], bf16, tag="zstr",
                                                  name="zch")
                        nc.sync.dma_start(zch[cix + 1][:, :], zd[cix + 1])
                    if cix - 2 in zch:
                        del zch[cix - 2]
                    base = 32 * (ts // 2)
                    sel = ia_bf if ts % 2 == 0 else ib_bf
                    zp = pspool.tile([16, G4], f32, tag="ps", name="rzp")
                    for nb in range(4):
                        nc.tensor.matmul(
                            zp[:, nb * 512:(nb + 1) * 512],
                            sel[base:base + 32, :],
                            zch[cix][base:base + 32, nb * 512:(nb + 1) * 512],
                            start=True, stop=st0, tile_position=(base, 0))
                    if not st0:
                        for kc in range(4):
                            for nb in range(4):
                                nc.tensor.matmul(
                                    zp[:, nb * 512:(nb + 1) * 512],
                                    hT_sb[:, kc, :],
                                    wh[:, kc, nb * 512:(nb + 1) * 512],
                                    start=False, stop=(kc == 3))
                    gates = gpool.tile([16, G4], f32, tag="gates", name="gates")
                    nc.scalar.activation(gates[:, 0:1536], zp[:, 0:1536], AF.Sigmoid)
                    nc.scalar.activation(gates[:, 1536:G4], zp[:, 1536:G4], AF.Tanh)
                    t1 = tpool.tile([16, 512], f32, tag="t1", name="t1")
                    t2 = tpool.tile([16, 512], f32, tag="t2", name="t2")
                    nc.vector.tensor_mul(t1[:, :], gates[:, 0:512], gates[:, 1536:G4])
                    nc.vector.tensor_mul(t2[:, :], gates[:, 512:1024], c_sb[:, :])
                    nc.vector.tensor_add(c_sb[:, :], t1[:, :], t2[:, :])
                    tcs = tpool.tile([16, 512], f32, tag="tc", name="tcs")
                    nc.scalar.activation(tcs[:, :], c_sb[:, :], AF.Tanh)
                    hs = tpool.tile([16, 512], f32, tag="h", name="hs")
                    nc.vector.tensor_mul(hs[:, :], gates[:, 1024:1536], tcs[:, :])
                    htp = pspool.tile([128, 4, 16], f32, tag="ps", name="htp")
                    for kc in range(4):
                        nc.tensor.matmul(
                            htp[:, kc, :], hs[:, kc * 128:(kc + 1) * 128],
                            i16_sb[:, :], start=(kc == 0), stop=(kc == 3),
                            is_transpose=True)
                    nc.vector.tensor_copy(hT_sb[:, :, :], htp[:, :, :])
                    if store_h1t:
                        nc.sync.dma_start(h1t[:, :, t, :], hT_sb[:, :, :])

            lstm_layer(z1d, wh_sb, store_h1t=True)

            # ================= Phase C: BN1 stats =================
            psum_parts = cpool.tile([128, 4, 4], f32, tag="p_sum")
            psq_parts = cpool.tile([128, 4, 4], f32, tag="p_sq")
            TCH = t_steps // 4  # stat chunk in timesteps
            for qi in reversed(range(4)):
                for kc in range(4):
                    hb = zpool.tile([128, TCH, 16], f32r, tag="zstr", name="hb")
                    nc.sync.dma_start(
                        hb[:, :, :], h1t[:, kc, qi * TCH:(qi + 1) * TCH, :])
                    hbv = hb[:, :, :].bitcast(f32)
                    tr1 = tpool.tile([128, TCH, 16], bf16, tag="trash", name="tr1")
                    nc.scalar.activation(tr1[:, :, :], hbv, AF.Identity,
                                         accum_out=psum_parts[:, kc, qi:qi + 1])
                    tr2 = tpool.tile([128, TCH, 16], bf16, tag="trash", name="tr2")
                    nc.scalar.activation(tr2[:, :, :], hbv, AF.Square,
                                         accum_out=psq_parts[:, kc, qi:qi + 1])
            allred = cpool.tile([128, 8], f32, tag="allred")
            nc.vector.tensor_reduce(allred[:, 0:4], psum_parts[:, :, :],
                                    mybir.AxisListType.X, mybir.AluOpType.add)
            nc.vector.tensor_reduce(allred[:, 4:8], psq_parts[:, :, :],
                                    mybir.AxisListType.X, mybir.AluOpType.add)
            nc.sync.dma_start(cc1_in[:, :], allred[:, :])
            nc.gpsimd.collective_compute(
                "AllReduce", mybir.AluOpType.add,
                replica_groups=[list(range(NCORES))],
                ins=[cc1_in.opt()], outs=[cc1_out.opt()])
            nc.sync.dma_start(allred[:, :], cc1_out[:, :])

            bn1s_sb = cpool.tile([128, 4], f32, tag="bn1s")
            bn1b_sb = cpool.tile([128, 4], f32, tag="bn1b")
            nc.sync.dma_start(bn1s_sb[:, :], d_bn1s[:, :])
            nc.sync.dma_start(bn1b_sb[:, :], d_bn1b[:, :])

            def bn_fold(allred_sb, n_count, bns, bnb):
                """Return (a, d): bn(x) = x*a + d per feature, [128,4] tiles."""
                mu = cpool.tile([128, 4], f32, tag=f"mu{n_count}")
                ex2 = cpool.tile([128, 4], f32, tag=f"ex2{n_count}")
                nc.vector.tensor_scalar_mul(mu[:, :], allred_sb[:, 0:4], 1.0 / n_count)
                nc.vector.tensor_scalar_mul(ex2[:, :], allred_sb[:, 4:8], 1.0 / n_count)
                var = cpool.tile([128, 4], f32, tag=f"var{n_count}")
                nc.vector.tensor_mul(var[:, :], mu[:, :], mu[:, :])
                nc.vector.tensor_sub(var[:, :], ex2[:, :], var[:, :])
                nc.vector.tensor_scalar_add(var[:, :], var[:, :], EPS)
                sd = cpool.tile([128, 4], f32, tag=f"sd{n_count}")
                nc.scalar.activation(sd[:, :], var[:, :], AF.Sqrt)
                r0 = cpool.tile([128, 4], f32, tag=f"r0{n_count}")
                nc.vector.reciprocal(r0[:, :], sd[:, :])
                # one Newton step: r1 = r0 * (1.5 - 0.5 * var * r0^2)
                e1 = cpool.tile([128, 4], f32, tag=f"e1{n_count}")
                nc.vector.tensor_mul(e1[:, :], r0[:, :], r0[:, :])
                nc.vector.tensor_mul(e1[:, :], e1[:, :], var[:, :])
                nc.vector.tensor_scalar(e1[:, :], e1[:, :], -0.5, 1.5,
                                        mybir.AluOpType.mult, mybir.AluOpType.add)
                nc.vector.tensor_mul(r0[:, :], r0[:, :], e1[:, :])
                a = cpool.tile([128, 4], f32, tag=f"a{n_count}")
                dv = cpool.tile([128, 4], f32, tag=f"d{n_count}")
                nc.vector.tensor_mul(a[:, :], r0[:, :], bns[:, :])
                nc.vector.tensor_mul(dv[:, :], mu[:, :], a[:, :])
                nc.vector.tensor_sub(dv[:, :], bnb[:, :], dv[:, :])
                return a, dv

            a1, d1v = bn_fold(allred, B * t_steps, bn1s_sb, bn1b_sb)

            # ================= Phase D: fold BN1 into Wi2 =================
            for kc in range(4):
                nc.sync.dma_start(wi_sb[:, kc, :], d_wi2[kc * 128:(kc + 1) * 128, :])
            b2_sb = cpool.tile([1, G4], f32r, tag="brow1")
            nc.sync.dma_start(b2_sb[:, :], d_b2[:, :])

            dvr = cpool.tile([128, 4], f32r, tag="dvr")
            nc.vector.tensor_copy(dvr[:, :], d1v[:, :])
            r2_ps = pspool.tile([1, G4], f32, tag="ps", name="r2ps")
            for kc in range(4):
                for nb in range(4):
                    nc.tensor.matmul(r2_ps[:, nb * 512:(nb + 1) * 512],
                                     dvr[:, kc:kc + 1],
                                     wi_sb[:, kc, nb * 512:(nb + 1) * 512],
                                     start=(kc == 0), stop=False)
            for nb in range(4):
                nc.tensor.matmul(r2_ps[:, nb * 512:(nb + 1) * 512],
                                 ones_sb[:, 0:1], b2_sb[:, nb * 512:(nb + 1) * 512],
                                 start=False, stop=True)
            r2_sb = cpool.tile([1, G4], f32r, tag="brow0r")
            nc.vector.tensor_copy(r2_sb[:, :], r2_ps[:, :])
            for kc in range(4):
                nc.vector.tensor_scalar_mul(wi_sb[:, kc, :], wi_sb[:, kc, :],
                                            a1[:, kc:kc + 1])

            # ================= Phase E: Z2 precompute =================
            def get_h1t_tile(c, kc):
                ht = xpool.tile([128, SPC, 16], f32r, tag="xt", name="htl")
                nc.sync.dma_start(ht[:, :, :],
                                  h1t[:, kc, c * SPC:(c + 1) * SPC, :])
                return ht

            z_precompute(z2d, get_h1t_tile, wi_sb, r2_sb)

            # ================= Phase F: L2 recurrence =================
            for kc in range(4):
                nc.sync.dma_start(wh_sb[:, kc, :], d_wh2[kc * 128:(kc + 1) * 128, :])
            lstm_layer(z2d, wh_sb, store_h1t=False)

            # ================= Phase G: BN2 + dense head =================
            hT32 = cpool.tile([128, 4, 16], f32, tag="hT32")
            nc.vector.tensor_copy(hT32[:, :, :], hT_sb[:, :, :])
            s2 = cpool.tile([128, 4], f32, tag="s2")
            q2 = cpool.tile([128, 4], f32, tag="q2")
            tr3 = cpool.tile([128, 4, 16], bf16, tag="tr3")
            for kc in range(4):
                nc.scalar.activation(tr3[:, kc, :], hT32[:, kc, :], AF.Identity,
                                     accum_out=s2[:, kc:kc + 1])
                nc.scalar.activation(tr3[:, kc, :], hT32[:, kc, :], AF.Square,
                                     accum_out=q2[:, kc:kc + 1])
            allred2 = cpool.tile([128, 8], f32, tag="allred2")
            nc.vector.tensor_copy(allred2[:, 0:4], s2[:, :])
            nc.vector.tensor_copy(allred2[:, 4:8], q2[:, :])
            nc.sync.dma_start(cc2_in[:, :], allred2[:, :])
            nc.gpsimd.collective_compute(
                "AllReduce", mybir.AluOpType.add,
                replica_groups=[list(range(NCORES))],
                ins=[cc2_in.opt()], outs=[cc2_out.opt()])
            nc.sync.dma_start(allred2[:, :], cc2_out[:, :])

            bn2s_sb = cpool.tile([128, 4], f32, tag="bn2s")
            bn2b_sb = cpool.tile([128, 4], f32, tag="bn2b")
            nc.sync.dma_start(bn2s_sb[:, :], d_bn2s[:, :])
            nc.sync.dma_start(bn2b_sb[:, :], d_bn2b[:, :])
            a2, d2v = bn_fold(allred2, B, bn2s_sb, bn2b_sb)

            wd1_sb = cpool.tile([128, 4, 16], f32, tag="wd1")
            for kc in range(4):
                nc.sync.dma_start(wd1_sb[:, kc, :], d_wd1[kc * 128:(kc + 1) * 128, :])
            bd1_sb = cpool.tile([16, 1], f32, tag="bd1")
            nc.sync.dma_start(bd1_sb[:, :], d_bd1[:, :])
            wd2_sb = cpool.tile([16, 1], f32, tag="wd2")
            nc.sync.dma_start(wd2_sb[:, :], d_wd2[:, :])
            bd2_sb = cpool.tile([1, 1], f32, tag="bd2")
            nc.sync.dma_start(bd2_sb[:, :], d_bd2[:, :])

            # bias_d1[j] = sum_h Wd1[h, j] * d2v[h] + bd1[j]  (psum [16, 1])
            bd1_ps = pspool.tile([16, 1], f32, tag="ps", name="bd1ps")
            for kc in range(4):
                nc.tensor.matmul(bd1_ps[:, :], wd1_sb[:, kc, :], d2v[:, kc:kc + 1],
                                 start=(kc == 0), stop=(kc == 3))
            biasd1 = cpool.tile([16, 1], f32, tag="biasd1")
            nc.vector.tensor_copy(biasd1[:, :], bd1_ps[:, :])
            nc.vector.tensor_add(biasd1[:, :], biasd1[:, :], bd1_sb[:, :])
            # scale Wd1 rows by a2 (after the bias matmuls read the raw Wd1)
            for kc in range(4):
                nc.vector.tensor_scalar_mul(wd1_sb[:, kc, :], wd1_sb[:, kc, :],
                                            a2[:, kc:kc + 1])
            # d1T[j, b] = tanh( sum_h Wd1'[h,j] * hT[h,b] + bias_d1[j] )
            d1_ps = pspool.tile([16, 16], f32, tag="ps", name="d1ps")
            for kc in range(4):
                nc.tensor.matmul(d1_ps[:, :], wd1_sb[:, kc, :], hT32[:, kc, :],
                                 start=(kc == 0), stop=(kc == 3))
            d1T = cpool.tile([16, 16], f32, tag="d1T")
            nc.scalar.activation(d1T[:, :], d1_ps[:, :], AF.Tanh, bias=biasd1[:, 0:1])
            # out[0, b] = sum_j Wd2[j] * d1T[j, b] + bd2
            o_ps = pspool.tile([1, 16], f32, tag="ps", name="ops")
            nc.tensor.matmul(o_ps[:, :], wd2_sb[:, :], d1T[:, :],
                             start=True, stop=True)
            out_sb = cpool.tile([1, 16], f32, tag="outsb")
            nc.scalar.activation(out_sb[:, :], o_ps[:, :], AF.Identity,
                                 bias=bd2_sb[:, 0:1])
            nc.sync.dma_start(d_out[:, :], out_sb[:, :])

    nc.compile()
    return nc


_PROG_CACHE = {}


def _get_program(t_steps):
    if t_steps not in _PROG_CACHE:
        _PROG_CACHE[t_steps] = _build_program(t_steps)
    return _PROG_CACHE[t_steps]


def kernel(x, Wi1, Wh1, b1, Wi2, Wh2, b2, bn1_scale, bn1_bias,
           bn2_scale, bn2_bias, Wd1, bd1, Wd2, bd2):
    from concourse.bass_utils import run_bass_kernel_spmd

    x = np.asarray(x, dtype=np.float32)
    t_steps = x.shape[1]
    nc = _get_program(t_steps)

    # gate reorder (i,f,g,o) -> (i,f,o,g)
    perm = np.concatenate([np.arange(0, 512), np.arange(512, 1024),
                           np.arange(1536, 2048), np.arange(1024, 1536)])
    wi1 = np.ascontiguousarray(np.asarray(Wi1, np.float32)[:, perm])
    wh1 = np.ascontiguousarray(np.asarray(Wh1, np.float32)[:, perm])
    b1p = np.asarray(b1, np.float32)[perm].reshape(1, G4)
    wi2 = np.ascontiguousarray(np.asarray(Wi2, np.float32)[:, perm])
    wh2 = np.ascontiguousarray(np.asarray(Wh2, np.float32)[:, perm])
    b2p = np.asarray(b2, np.float32)[perm].reshape(1, G4)

    def col4(v):
        return np.ascontiguousarray(np.asarray(v, np.float32).reshape(4, 128).T)

    ia = np.zeros((128, 16), np.float32)
    ib = np.zeros((128, 16), np.float32)
    for g in range(4):
        for j in range(16):
            ia[32 * g + j, j] = 1.0
            ib[32 * g + 16 + j, j] = 1.0
    common = {
        "wi1": wi1, "wh1": wh1, "b1row": b1p,
        "wi2": wi2, "wh2": wh2, "b2row": b2p,
        "bn1s": col4(bn1_scale), "bn1b": col4(bn1_bias),
        "bn2s": col4(bn2_scale), "bn2b": col4(bn2_bias),
        "wd1": np.asarray(Wd1, np.float32),
        "bd1c": np.asarray(bd1, np.float32).reshape(16, 1),
        "wd2": np.asarray(Wd2, np.float32).reshape(16, 1),
        "bd2c": np.asarray(bd2, np.float32).reshape(1, 1),
        "IA": ia, "IB": ib, "I16": np.eye(16, dtype=np.float32),
        "ones1": np.ones((1, 128), np.float32),
    }
    in_maps = []
    for ci in range(NCORES):
        xs = x[ci * BL:(ci + 1) * BL]                    # [16, T, F]
        xT = np.ascontiguousarray(xs.transpose(2, 1, 0).reshape(F, t_steps * BL))
        m = dict(common)
        m["xT"] = xT
        in_maps.append(m)

    global _LAST_IN_MAPS
    _LAST_IN_MAPS = in_maps
    res = run_bass_kernel_spmd(nc, in_maps, core_ids=list(range(NCORES)))
    y = np.concatenate(
        [res.results[ci]["out"].reshape(16, 1) for ci in range(NCORES)], axis=0)
    return y.astype(np.float32)


# revision 25
# speedup vs baseline: 414.8648x; 1.0571x over previous
"""Trainium2 Bass kernel for nn_LSTMSimple: 2-layer LSTM + BatchNorm + dense head.

Strategy: data-parallel over batch (128 -> 16 per core, 8 cores).
Per core:
  A) Z1 = X @ Wi1 + b1 precomputed for all timesteps (PE, big matmuls) -> HBM
  B) L1 recurrence: per step z = Z1[t] + h @ Wh1 (Z1[t] injected into the PSUM
     accumulation group via an identity-stationary matmul), sigmoid/tanh on
     ScalarE, c/h updates on VectorE, h -> h^T via 4 PE transpose matmuls.
     h^T also streamed to HBM (it is the stationary operand of the Z2 matmul).
  C) BN1 batch stats via ScalarE accum_out + one 4KB AllReduce; BN1 is folded
     into the Z2 precompute (scale rows of Wi2, add a bias row) - the
     normalized activations are never materialized.
  E) Z2 = H1bn @ Wi2 + b2 precompute from stored h^T tiles.
  F) L2 recurrence (identical, no state store; keeps final h^T).
  G) BN2 stats AllReduce, folded into Wd1; dense head on PE; out = [1, 16].
Host reorders gate columns from (i,f,g,o) to (i,f,o,g) so one sigmoid op
covers columns 0:1536 and one tanh op covers 1536:2048.

v5 over the baseline: large matmuls run at 1 cycle/row instead of 4 --
weights/x/h^T tensors are declared native float32r end-to-end (DMA-produced
or TensorCopy-produced, per the BIR verifier's rounding rule), and the Z
stream (x@Wi+b) is stored/injected in bf16. Structure, PSUM layout and
engine choreography are identical to the baseline (all constructs
hardware-proven); only dtypes and the Z staging changed.
"""

import sys

if '/opt/trn_rl_repo' not in sys.path:
    sys.path.insert(0, '/opt/trn_rl_repo')

import numpy as np

# ---- problem constants (hardcoded per contract) ----
B = 128
T = int(__import__('os').environ.get('LSTM_T', '512'))  # debug knob
F = 512
H = 512
G4 = 4 * H           # 2048
NCORES = 8
BL = B // NCORES     # 16 batch rows per core
SPC = 8              # timesteps per Z chunk (128 = 8*16 partition rows)
EPS = 1e-5


def _build_program(t_steps: int):
    import concourse.bacc as bacc
    import concourse.mybir as mybir
    import concourse.tile as tile

    f32 = mybir.dt.float32
    f32r = mybir.dt.float32r
    bf16 = mybir.dt.bfloat16
    AF = mybir.ActivationFunctionType

    NCH = t_steps // SPC  # z chunks per layer

    nc = bacc.Bacc("TRN2", target_bir_lowering=False, debug=False,
                   num_devices=NCORES)

    # ---- kernel I/O ----
    d_xT = nc.dram_tensor("xT", [F, t_steps * BL], f32r, kind="ExternalInput")
    d_wi1 = nc.dram_tensor("wi1", [F, G4], f32r, kind="ExternalInput")
    d_wh1 = nc.dram_tensor("wh1", [H, G4], f32r, kind="ExternalInput")
    d_b1 = nc.dram_tensor("b1row", [1, G4], f32r, kind="ExternalInput")
    d_wi2 = nc.dram_tensor("wi2", [H, G4], f32r, kind="ExternalInput")
    d_wh2 = nc.dram_tensor("wh2", [H, G4], f32r, kind="ExternalInput")
    d_b2 = nc.dram_tensor("b2row", [1, G4], f32r, kind="ExternalInput")
    d_bn1s = nc.dram_tensor("bn1s", [128, 4], f32, kind="ExternalInput")
    d_bn1b = nc.dram_tensor("bn1b", [128, 4], f32, kind="ExternalInput")
    d_bn2s = nc.dram_tensor("bn2s", [128, 4], f32, kind="ExternalInput")
    d_bn2b = nc.dram_tensor("bn2b", [128, 4], f32, kind="ExternalInput")
    d_wd1 = nc.dram_tensor("wd1", [H, 16], f32, kind="ExternalInput")
    d_bd1 = nc.dram_tensor("bd1c", [16, 1], f32, kind="ExternalInput")
    d_wd2 = nc.dram_tensor("wd2", [16, 1], f32, kind="ExternalInput")
    d_bd2 = nc.dram_tensor("bd2c", [1, 1], f32, kind="ExternalInput")
    d_ia = nc.dram_tensor("IA", [128, 16], f32, kind="ExternalInput")
    d_ib = nc.dram_tensor("IB", [128, 16], f32, kind="ExternalInput")
    d_i16 = nc.dram_tensor("I16", [16, 16], f32, kind="ExternalInput")
    d_ones = nc.dram_tensor("ones1", [1, 128], f32r, kind="ExternalInput")
    d_out = nc.dram_tensor("out", [1, 16], f32, kind="ExternalOutput")

    with tile.TileContext(nc) as tc:
        with (
            tc.tile_pool(name="const", bufs=1) as cpool,
            tc.tile_pool(name="wpool", bufs=1) as wpool,
            tc.tile_pool(name="zstr", bufs=3) as zpool,
            tc.tile_pool(name="xt", bufs=8) as xpool,
            tc.tile_pool(name="gat", bufs=2) as gpool,
            tc.tile_pool(name="tmp", bufs=2) as tpool,
            tc.tile_pool(name="ps", bufs=2, space="PSUM") as pspool,
            tc.tile_pool(name="dram", bufs=1, space="DRAM") as dpool,
        ):
            # ---- constants / weights in SBUF ----
            ia_sb = cpool.tile([128, 16], f32, tag="ia")
            ib_sb = cpool.tile([128, 16], f32, tag="ib")
            i16_sb = cpool.tile([16, 16], f32, tag="i16")
            ones_sb = cpool.tile([1, 128], f32r, tag="ones")
            nc.sync.dma_start(ia_sb[:, :], d_ia[:, :])
            nc.sync.dma_start(ib_sb[:, :], d_ib[:, :])
            nc.sync.dma_start(i16_sb[:, :], d_i16[:, :])
            nc.sync.dma_start(ones_sb[:, :], d_ones[:, :])
            ia_bf = cpool.tile([128, 16], bf16, tag="iabf")
            ib_bf = cpool.tile([128, 16], bf16, tag="ibbf")
            nc.vector.tensor_copy(ia_bf[:, :], ia_sb[:, :])
            nc.vector.tensor_copy(ib_bf[:, :], ib_sb[:, :])

            wi_sb = wpool.tile([128, 4, G4], f32r, tag="wi")  # Wi1, later Wi2
            for kc in range(4):
                nc.sync.dma_start(wi_sb[:, kc, :], d_wi1[kc * 128:(kc + 1) * 128, :])
            b1_sb = cpool.tile([1, G4], f32r, tag="brow0")
            nc.sync.dma_start(b1_sb[:, :], d_b1[:, :])

            wh_sb = wpool.tile([128, 4, G4], f32r, tag="wh")  # Wh1, later Wh2
            for kc in range(4):
                nc.sync.dma_start(wh_sb[:, kc, :], d_wh1[kc * 128:(kc + 1) * 128, :])

            # ---- DRAM intermediates ----
            z1d = dpool.tile([NCH, 128, G4], bf16, tag="z1d")
            z2d = dpool.tile([NCH, 128, G4], bf16, tag="z2d")
            h1t = dpool.tile([128, 4, t_steps, 16], f32r, tag="h1t")
            cc1_in = dpool.tile([128, 8], f32, tag="cc1i")
            cc1_out = dpool.tile([128, 8], f32, tag="cc1o")
            cc2_in = dpool.tile([128, 8], f32, tag="cc2i")
            cc2_out = dpool.tile([128, 8], f32, tag="cc2o")

            # ================= Phase A: Z1 precompute =================
            def z_precompute(zd, get_lhs_tile, rhs_w, bias_row):
                """zd[c] = lhsT_c.T @ W + bias_row for all row chunks (bf16)."""
                for c in range(NCH):
                    lhs = [get_lhs_tile(c, kc) for kc in range(4)]
                    zp = pspool.tile([128, G4], f32, tag="ps", name="zp")
                    for kc in range(4):
                        for nb in range(4):
                            nc.tensor.matmul(
                                zp[:, nb * 512:(nb + 1) * 512],
                                lhs[kc][:, :],
                                rhs_w[:, kc, nb * 512:(nb + 1) * 512],
                                start=(kc == 0), stop=False)
                    for nb in range(4):
                        nc.tensor.matmul(
                            zp[:, nb * 512:(nb + 1) * 512],
                            ones_sb[:, :],
                            bias_row[:, nb * 512:(nb + 1) * 512],
                            start=False, stop=True)
                    zsb = zpool.tile([128, G4], bf16, tag="zstr", name="zsb")
                    nc.vector.tensor_copy(zsb[:, :], zp[:, :])
                    nc.sync.dma_start(zd[c], zsb[:, :])

            def get_x_tile(c, kc):
                xt = xpool.tile([128, 128], f32r, tag="xt", name="xt")
                nc.sync.dma_start(
                    xt[:, :], d_xT[kc * 128:(kc + 1) * 128, c * 128:(c + 1) * 128])
                return xt

            z_precompute(z1d, get_x_tile, wi_sb, b1_sb)

            # ---- persistent recurrence state ----
            hT_sb = cpool.tile([128, 4, 16], f32r, tag="hT")
            c_sb = cpool.tile([16, 512], bf16, tag="cst")

            # ================= recurrence =================
            def lstm_layer(zd, wh, store_h1t):
                nc.vector.memset(c_sb[:, :], 0.0)
                zch = {0: zpool.tile([128, G4], bf16, tag="zstr", name="zch0")}
                nc.sync.dma_start(zch[0][:, :], zd[0])
                for t in range(t_steps):
                    st0 = (t == 0)  # h == 0: skip the Wh matmuls
                    cix, ts = divmod(t, SPC)
                    if ts == 0 and cix + 1 < NCH:
                        zch[cix + 1] = zpool.tile([128, G4], bf16, tag="zstr",
                                                  name="zch")
                        nc.sync.dma_start(zch[cix + 1][:, :], zd[cix + 1])
                    if cix - 2 in zch:
                        del zch[cix - 2]
                    base = 32 * (ts // 2)
                    sel = ia_bf if ts % 2 == 0 else ib_bf
                    zp = pspool.tile([16, G4], f32, tag="ps", name="rzp")
                    for nb in range(4):
                        nc.tensor.matmul(
                            zp[:, nb * 512:(nb + 1) * 512],
                            sel[base:base + 32, :],
                            zch[cix][base:base + 32, nb * 512:(nb + 1) * 512],
                            start=True, stop=st0, tile_position=(base, 0))
                    if not st0:
                        for kc in range(4):
                            for nb in range(4):
                                nc.tensor.matmul(
                                    zp[:, nb * 512:(nb + 1) * 512],
                                    hT_sb[:, kc, :],
                                    wh[:, kc, nb * 512:(nb + 1) * 512],
                                    start=False, stop=(kc == 3))
                    # gates/c in bf16: DVE gets the 2x packed mode; PSUM
                    # accumulation and h stay fp32. sigmoid(i,f) first so the
                    # c-chain starts while ACT still does tanh(g)/sigmoid(o).
                    gates = gpool.tile([16, G4], bf16, tag="gates", name="gates")
                    nc.scalar.activation(gates[:, 0:1024], zp[:, 0:1024],
                                         AF.Sigmoid)
                    nc.scalar.activation(gates[:, 1536:G4], zp[:, 1536:G4],
                                         AF.Tanh)
                    t2 = tpool.tile([16, 512], bf16, tag="t2", name="t2")
                    nc.vector.tensor_mul(t2[:, :], gates[:, 512:1024], c_sb[:, :])
                    nc.scalar.activation(gates[:, 1024:1536], zp[:, 1024:1536],
                                         AF.Sigmoid)
                    t1 = tpool.tile([16, 512], bf16, tag="t1", name="t1")
                    nc.vector.tensor_mul(t1[:, :], gates[:, 0:512], gates[:, 1536:G4])
                    nc.vector.tensor_add(c_sb[:, :], t1[:, :], t2[:, :])
                    tcs = tpool.tile([16, 512], bf16, tag="tc", name="tcs")
                    nc.scalar.activation(tcs[:, :], c_sb[:, :], AF.Tanh)
                    hs = tpool.tile([16, 512], f32, tag="h", name="hs")
                    nc.vector.tensor_mul(hs[:, :], gates[:, 1024:1536], tcs[:, :])
                    htp = pspool.tile([128, 4, 16], f32, tag="ps", name="htp")
                    for kc in range(4):
                        nc.tensor.matmul(
                            htp[:, kc, :], hs[:, kc * 128:(kc + 1) * 128],
                            i16_sb[:, :], start=(kc == 0), stop=(kc == 3),
                            is_transpose=True)
                    nc.vector.tensor_copy(hT_sb[:, :, :], htp[:, :, :])
                    if store_h1t:
                        nc.sync.dma_start(h1t[:, :, t, :], hT_sb[:, :, :])

            lstm_layer(z1d, wh_sb, store_h1t=True)

            # ================= Phase C: BN1 stats =================
            psum_parts = cpool.tile([128, 4, 4], f32, tag="p_sum")
            psq_parts = cpool.tile([128, 4, 4], f32, tag="p_sq")
            TCH = t_steps // 4  # stat chunk in timesteps
            for qi in reversed(range(4)):
                for kc in range(4):
                    hb = zpool.tile([128, TCH, 16], f32r, tag="zstr", name="hb")
                    nc.sync.dma_start(
                        hb[:, :, :], h1t[:, kc, qi * TCH:(qi + 1) * TCH, :])
                    hbv = hb[:, :, :].bitcast(f32)
                    tr1 = tpool.tile([128, TCH, 16], bf16, tag="trash", name="tr1")
                    nc.scalar.activation(tr1[:, :, :], hbv, AF.Identity,
                                         accum_out=psum_parts[:, kc, qi:qi + 1])
                    tr2 = tpool.tile([128, TCH, 16], bf16, tag="trash", name="tr2")
                    nc.scalar.activation(tr2[:, :, :], hbv, AF.Square,
                                         accum_out=psq_parts[:, kc, qi:qi + 1])
            allred = cpool.tile([128, 8], f32, tag="allred")
            nc.vector.tensor_reduce(allred[:, 0:4], psum_parts[:, :, :],
                                    mybir.AxisListType.X, mybir.AluOpType.add)
            nc.vector.tensor_reduce(allred[:, 4:8], psq_parts[:, :, :],
                                    mybir.AxisListType.X, mybir.AluOpType.add)
            nc.sync.dma_start(cc1_in[:, :], allred[:, :])
            nc.gpsimd.collective_compute(
                "AllReduce", mybir.AluOpType.add,
                replica_groups=[list(range(NCORES))],
                ins=[cc1_in.opt()], outs=[cc1_out.opt()])
            nc.sync.dma_start(allred[:, :], cc1_out[:, :])

            bn1s_sb = cpool.tile([128, 4], f32, tag="bn1s")
            bn1b_sb = cpool.tile([128, 4], f32, tag="bn1b")
            nc.sync.dma_start(bn1s_sb[:, :], d_bn1s[:, :])
            nc.sync.dma_start(bn1b_sb[:, :], d_bn1b[:, :])

            def bn_fold(allred_sb, n_count, bns, bnb):
                """Return (a, d): bn(x) = x*a + d per feature, [128,4] tiles."""
                mu = cpool.tile([128, 4], f32, tag=f"mu{n_count}")
                ex2 = cpool.tile([128, 4], f32, tag=f"ex2{n_count}")
                nc.vector.tensor_scalar_mul(mu[:, :], allred_sb[:, 0:4], 1.0 / n_count)
                nc.vector.tensor_scalar_mul(ex2[:, :], allred_sb[:, 4:8], 1.0 / n_count)
                var = cpool.tile([128, 4], f32, tag=f"var{n_count}")
                nc.vector.tensor_mul(var[:, :], mu[:, :], mu[:, :])
                nc.vector.tensor_sub(var[:, :], ex2[:, :], var[:, :])
                nc.vector.tensor_scalar_add(var[:, :], var[:, :], EPS)
                sd = cpool.tile([128, 4], f32, tag=f"sd{n_count}")
                nc.scalar.activation(sd[:, :], var[:, :], AF.Sqrt)
                r0 = cpool.tile([128, 4], f32, tag=f"r0{n_count}")
                nc.vector.reciprocal(r0[:, :], sd[:, :])
                # one Newton step: r1 = r0 * (1.5 - 0.5 * var * r0^2)
                e1 = cpool.tile([128, 4], f32, tag=f"e1{n_count}")
                nc.vector.tensor_mul(e1[:, :], r0[:, :], r0[:, :])
                nc.vector.tensor_mul(e1[:, :], e1[:, :], var[:, :])
                nc.vector.tensor_scalar(e1[:, :], e1[:, :], -0.5, 1.5,
                                        mybir.AluOpType.mult, mybir.AluOpType.add)
                nc.vector.tensor_mul(r0[:, :], r0[:, :], e1[:, :])
                a = cpool.tile([128, 4], f32, tag=f"a{n_count}")
                dv = cpool.tile([128, 4], f32, tag=f"d{n_count}")
                nc.vector.tensor_mul(a[:, :], r0[:, :], bns[:, :])
                nc.vector.tensor_mul(dv[:, :], mu[:, :], a[:, :])
                nc.vector.tensor_sub(dv[:, :], bnb[:, :], dv[:, :])
                return a, dv

            a1, d1v = bn_fold(allred, B * t_steps, bn1s_sb, bn1b_sb)

            # ================= Phase D: fold BN1 into Wi2 =================
            for kc in range(4):
                nc.sync.dma_start(wi_sb[:, kc, :], d_wi2[kc * 128:(kc + 1) * 128, :])
            b2_sb = cpool.tile([1, G4], f32r, tag="brow1")
            nc.sync.dma_start(b2_sb[:, :], d_b2[:, :])

            dvr = cpool.tile([128, 4], f32r, tag="dvr")
            nc.vector.tensor_copy(dvr[:, :], d1v[:, :])
            r2_ps = pspool.tile([1, G4], f32, tag="ps", name="r2ps")
            for kc in range(4):
                for nb in range(4):
                    nc.tensor.matmul(r2_ps[:, nb * 512:(nb + 1) * 512],
                                     dvr[:, kc:kc + 1],
                                     wi_sb[:, kc, nb * 512:(nb + 1) * 512],
                                     start=(kc == 0), stop=False)
            for nb in range(4):
                nc.tensor.matmul(r2_ps[:, nb * 512:(nb + 1) * 512],
                                 ones_sb[:, 0:1], b2_sb[:, nb * 512:(nb + 1) * 512],
                                 start=False, stop=True)
            r2_sb = cpool.tile([1, G4], f32r, tag="brow0r")
            nc.vector.tensor_copy(r2_sb[:, :], r2_ps[:, :])
            for kc in range(4):
                nc.vector.tensor_scalar_mul(wi_sb[:, kc, :], wi_sb[:, kc, :],
                                            a1[:, kc:kc + 1])

            # ================= Phase E: Z2 precompute =================
            def get_h1t_tile(c, kc):
                ht = xpool.tile([128, SPC, 16], f32r, tag="xt", name="htl")
                nc.sync.dma_start(ht[:, :, :],
                                  h1t[:, kc, c * SPC:(c + 1) * SPC, :])
                return ht

            z_precompute(z2d, get_h1t_tile, wi_sb, r2_sb)

            # ================= Phase F: L2 recurrence =================
            for kc in range(4):
                nc.sync.dma_start(wh_sb[:, kc, :], d_wh2[kc * 128:(kc + 1) * 128, :])
            lstm_layer(z2d, wh_sb, store_h1t=False)

            # ================= Phase G: BN2 + dense head =================
            hT32 = cpool.tile([128, 4, 16], f32, tag="hT32")
            nc.vector.tensor_copy(hT32[:, :, :], hT_sb[:, :, :])
            s2 = cpool.tile([128, 4], f32, tag="s2")
            q2 = cpool.tile([128, 4], f32, tag="q2")
            tr3 = cpool.tile([128, 4, 16], bf16, tag="tr3")
            for kc in range(4):
                nc.scalar.activation(tr3[:, kc, :], hT32[:, kc, :], AF.Identity,
                                     accum_out=s2[:, kc:kc + 1])
                nc.scalar.activation(tr3[:, kc, :], hT32[:, kc, :], AF.Square,
                                     accum_out=q2[:, kc:kc + 1])
            allred2 = cpool.tile([128, 8], f32, tag="allred2")
            nc.vector.tensor_copy(allred2[:, 0:4], s2[:, :])
            nc.vector.tensor_copy(allred2[:, 4:8], q2[:, :])
            nc.sync.dma_start(cc2_in[:, :], allred2[:, :])
            nc.gpsimd.collective_compute(
                "AllReduce", mybir.AluOpType.add,
                replica_groups=[list(range(NCORES))],
                ins=[cc2_in.opt()], outs=[cc2_out.opt()])
            nc.sync.dma_start(allred2[:, :], cc2_out[:, :])

            bn2s_sb = cpool.tile([128, 4], f32, tag="bn2s")
            bn2b_sb = cpool.tile([128, 4], f32, tag="bn2b")
            nc.sync.dma_start(bn2s_sb[:, :], d_bn2s[:, :])
            nc.sync.dma_start(bn2b_sb[:, :], d_bn2b[:, :])
            a2, d2v = bn_fold(allred2, B, bn2s_sb, bn2b_sb)

            wd1_sb = cpool.tile([128, 4, 16], f32, tag="wd1")
            for kc in range(4):
                nc.sync.dma_start(wd1_sb[:, kc, :], d_wd1[kc * 128:(kc + 1) * 128, :])
            bd1_sb = cpool.tile([16, 1], f32, tag="bd1")
            nc.sync.dma_start(bd1_sb[:, :], d_bd1[:, :])
            wd2_sb = cpool.tile([16, 1], f32, tag="wd2")
            nc.sync.dma_start(wd2_sb[:, :], d_wd2[:, :])
            bd2_sb = cpool.tile([1, 1], f32, tag="bd2")
            nc.sync.dma_start(bd2_sb[:, :], d_bd2[:, :])

            # bias_d1[j] = sum_h Wd1[h, j] * d2v[h] + bd1[j]  (psum [16, 1])
            bd1_ps = pspool.tile([16, 1], f32, tag="ps", name="bd1ps")
            for kc in range(4):
                nc.tensor.matmul(bd1_ps[:, :], wd1_sb[:, kc, :], d2v[:, kc:kc + 1],
                                 start=(kc == 0), stop=(kc == 3))
            biasd1 = cpool.tile([16, 1], f32, tag="biasd1")
            nc.vector.tensor_copy(biasd1[:, :], bd1_ps[:, :])
            nc.vector.tensor_add(biasd1[:, :], biasd1[:, :], bd1_sb[:, :])
            # scale Wd1 rows by a2 (after the bias matmuls read the raw Wd1)
            for kc in range(4):
                nc.vector.tensor_scalar_mul(wd1_sb[:, kc, :], wd1_sb[:, kc, :],
                                            a2[:, kc:kc + 1])
            # d1T[j, b] = tanh( sum_h Wd1'[h,j] * hT[h,b] + bias_d1[j] )
            d1_ps = pspool.tile([16, 16], f32, tag="ps", name="d1ps")
            for kc in range(4):
                nc.tensor.matmul(d1_ps[:, :], wd1_sb[:, kc, :], hT32[:, kc, :],
                                 start=(kc == 0), stop=(kc == 3))
            d1T = cpool.tile([16, 16], f32, tag="d1T")
            nc.scalar.activation(d1T[:, :], d1_ps[:, :], AF.Tanh, bias=biasd1[:, 0:1])
            # out[0, b] = sum_j Wd2[j] * d1T[j, b] + bd2
            o_ps = pspool.tile([1, 16], f32, tag="ps", name="ops")
            nc.tensor.matmul(o_ps[:, :], wd2_sb[:, :], d1T[:, :],
                             start=True, stop=True)
            out_sb = cpool.tile([1, 16], f32, tag="outsb")
            nc.scalar.activation(out_sb[:, :], o_ps[:, :], AF.Identity,
                                 bias=bd2_sb[:, 0:1])
            nc.sync.dma_start(d_out[:, :], out_sb[:, :])

    nc.compile()
    return nc


_PROG_CACHE = {}


def _get_program(t_steps):
    if t_steps not in _PROG_CACHE:
        _PROG_CACHE[t_steps] = _build_program(t_steps)
    return _PROG_CACHE[t_steps]


def kernel(x, Wi1, Wh1, b1, Wi2, Wh2, b2, bn1_scale, bn1_bias,
           bn2_scale, bn2_bias, Wd1, bd1, Wd2, bd2):
    from concourse.bass_utils import run_bass_kernel_spmd

    x = np.asarray(x, dtype=np.float32)
    t_steps = x.shape[1]
    nc = _get_program(t_steps)

    # gate reorder (i,f,g,o) -> (i,f,o,g)
    perm = np.concatenate([np.arange(0, 512), np.arange(512, 1024),
                           np.arange(1536, 2048), np.arange(1024, 1536)])
    wi1 = np.ascontiguousarray(np.asarray(Wi1, np.float32)[:, perm])
    wh1 = np.ascontiguousarray(np.asarray(Wh1, np.float32)[:, perm])
    b1p = np.asarray(b1, np.float32)[perm].reshape(1, G4)
    wi2 = np.ascontiguousarray(np.asarray(Wi2, np.float32)[:, perm])
    wh2 = np.ascontiguousarray(np.asarray(Wh2, np.float32)[:, perm])
    b2p = np.asarray(b2, np.float32)[perm].reshape(1, G4)

    def col4(v):
        return np.ascontiguousarray(np.asarray(v, np.float32).reshape(4, 128).T)

    ia = np.zeros((128, 16), np.float32)
    ib = np.zeros((128, 16), np.float32)
    for g in range(4):
        for j in range(16):
            ia[32 * g + j, j] = 1.0
            ib[32 * g + 16 + j, j] = 1.0
    common = {
        "wi1": wi1, "wh1": wh1, "b1row": b1p,
        "wi2": wi2, "wh2": wh2, "b2row": b2p,
        "bn1s": col4(bn1_scale), "bn1b": col4(bn1_bias),
        "bn2s": col4(bn2_scale), "bn2b": col4(bn2_bias),
        "wd1": np.asarray(Wd1, np.float32),
        "bd1c": np.asarray(bd1, np.float32).reshape(16, 1),
        "wd2": np.asarray(Wd2, np.float32).reshape(16, 1),
        "bd2c": np.asarray(bd2, np.float32).reshape(1, 1),
        "IA": ia, "IB": ib, "I16": np.eye(16, dtype=np.float32),
        "ones1": np.ones((1, 128), np.float32),
    }
    in_maps = []
    for ci in range(NCORES):
        xs = x[ci * BL:(ci + 1) * BL]                    # [16, T, F]
        xT = np.ascontiguousarray(xs.transpose(2, 1, 0).reshape(F, t_steps * BL))
        m = dict(common)
        m["xT"] = xT
        in_maps.append(m)

    global _LAST_IN_MAPS
    _LAST_IN_MAPS = in_maps
    res = run_bass_kernel_spmd(nc, in_maps, core_ids=list(range(NCORES)))
    y = np.concatenate(
        [res.results[ci]["out"].reshape(16, 1) for ci in range(NCORES)], axis=0)
    return y.astype(np.float32)


# revision 26
# speedup vs baseline: 699.5027x; 1.6861x over previous
"""Trainium2 Bass kernel for nn_LSTMSimple: 2-layer LSTM + BatchNorm + dense head.

Strategy: data-parallel over batch (128 -> 16 per core, 8 cores).
Per core:
  A) Z1 = X @ Wi1 + b1 precomputed for all timesteps (PE, big matmuls) -> HBM
  B) L1 recurrence: per step z = Z1[t] + h @ Wh1 (Z1[t] injected into the PSUM
     accumulation group via an identity-stationary matmul), sigmoid/tanh on
     ScalarE, c/h updates on VectorE, h -> h^T via 4 PE transpose matmuls.
     h^T also streamed to HBM (it is the stationary operand of the Z2 matmul).
  C) BN1 batch stats via ScalarE accum_out + one 4KB AllReduce; BN1 is folded
     into the Z2 precompute (scale rows of Wi2, add a bias row) - the
     normalized activations are never materialized.
  E) Z2 = H1bn @ Wi2 + b2 precompute from stored h^T tiles.
  F) L2 recurrence (identical, no state store; keeps final h^T).
  G) BN2 stats AllReduce, folded into Wd1; dense head on PE; out = [1, 16].
Host reorders gate columns from (i,f,g,o) to (i,f,o,g) so one sigmoid op
covers columns 0:1536 and one tanh op covers 1536:2048.

v5 over the baseline: large matmuls run at 1 cycle/row instead of 4 --
weights/x/h^T tensors are declared native float32r end-to-end (DMA-produced
or TensorCopy-produced, per the BIR verifier's rounding rule), and the Z
stream (x@Wi+b) is stored/injected in bf16. Structure, PSUM layout and
engine choreography are identical to the baseline (all constructs
hardware-proven); only dtypes and the Z staging changed.
"""

import sys

if '/opt/trn_rl_repo' not in sys.path:
    sys.path.insert(0, '/opt/trn_rl_repo')

import numpy as np

# ---- problem constants (hardcoded per contract) ----
B = 128
T = int(__import__('os').environ.get('LSTM_T', '512'))  # debug knob
F = 512
H = 512
G4 = 4 * H           # 2048
NCORES = 8
BL = B // NCORES     # 16 batch rows per core
SPC = 8              # timesteps per Z chunk (128 = 8*16 partition rows)
EPS = 1e-5


def _build_program(t_steps: int):
    import concourse.bacc as bacc
    import concourse.mybir as mybir
    import concourse.tile as tile

    f32 = mybir.dt.float32
    f32r = mybir.dt.float32r
    bf16 = mybir.dt.bfloat16
    AF = mybir.ActivationFunctionType

    NCH = t_steps // SPC  # z chunks per layer

    nc = bacc.Bacc("TRN2", target_bir_lowering=False, debug=False,
                   num_devices=NCORES)

    # ---- kernel I/O ----
    d_xT = nc.dram_tensor("xT", [F, t_steps * BL], f32r, kind="ExternalInput")
    d_wi1 = nc.dram_tensor("wi1", [F, G4], f32r, kind="ExternalInput")
    d_wh1 = nc.dram_tensor("wh1", [H, G4], f32r, kind="ExternalInput")
    d_b1 = nc.dram_tensor("b1row", [1, G4], f32r, kind="ExternalInput")
    d_wi2 = nc.dram_tensor("wi2", [H, G4], f32r, kind="ExternalInput")
    d_wh2 = nc.dram_tensor("wh2", [H, G4], f32r, kind="ExternalInput")
    d_b2 = nc.dram_tensor("b2row", [1, G4], f32r, kind="ExternalInput")
    d_bn1s = nc.dram_tensor("bn1s", [128, 4], f32, kind="ExternalInput")
    d_bn1b = nc.dram_tensor("bn1b", [128, 4], f32, kind="ExternalInput")
    d_bn2s = nc.dram_tensor("bn2s", [128, 4], f32, kind="ExternalInput")
    d_bn2b = nc.dram_tensor("bn2b", [128, 4], f32, kind="ExternalInput")
    d_wd1 = nc.dram_tensor("wd1", [H, 16], f32, kind="ExternalInput")
    d_bd1 = nc.dram_tensor("bd1c", [16, 1], f32, kind="ExternalInput")
    d_wd2 = nc.dram_tensor("wd2", [16, 1], f32, kind="ExternalInput")
    d_bd2 = nc.dram_tensor("bd2c", [1, 1], f32, kind="ExternalInput")
    d_ia = nc.dram_tensor("IA", [128, 16], f32, kind="ExternalInput")
    d_ib = nc.dram_tensor("IB", [128, 16], f32, kind="ExternalInput")
    d_i16 = nc.dram_tensor("I16", [16, 16], f32, kind="ExternalInput")
    d_ones = nc.dram_tensor("ones1", [1, 128], f32r, kind="ExternalInput")
    d_out = nc.dram_tensor("out", [1, 16], f32, kind="ExternalOutput")

    with tile.TileContext(nc) as tc:
        with (
            tc.tile_pool(name="const", bufs=1) as cpool,
            tc.tile_pool(name="wpool", bufs=1) as wpool,
            tc.tile_pool(name="zstr", bufs=3) as zpool,
            tc.tile_pool(name="xt", bufs=8) as xpool,
            tc.tile_pool(name="gat", bufs=2) as gpool,
            tc.tile_pool(name="tmp", bufs=2) as tpool,
            tc.tile_pool(name="ps", bufs=2, space="PSUM") as pspool,
            tc.tile_pool(name="dram", bufs=1, space="DRAM") as dpool,
        ):
            # ---- constants / weights in SBUF ----
            ia_sb = cpool.tile([128, 16], f32, tag="ia")
            ib_sb = cpool.tile([128, 16], f32, tag="ib")
            i16_sb = cpool.tile([16, 16], f32, tag="i16")
            ones_sb = cpool.tile([1, 128], f32r, tag="ones")
            nc.sync.dma_start(ia_sb[:, :], d_ia[:, :])
            nc.sync.dma_start(ib_sb[:, :], d_ib[:, :])
            nc.sync.dma_start(i16_sb[:, :], d_i16[:, :])
            nc.sync.dma_start(ones_sb[:, :], d_ones[:, :])
            ia_bf = cpool.tile([128, 16], bf16, tag="iabf")
            ib_bf = cpool.tile([128, 16], bf16, tag="ibbf")
            nc.vector.tensor_copy(ia_bf[:, :], ia_sb[:, :])
            nc.vector.tensor_copy(ib_bf[:, :], ib_sb[:, :])

            wi_sb = wpool.tile([128, 4, G4], f32r, tag="wi")  # Wi1, later Wi2
            for kc in range(4):
                nc.sync.dma_start(wi_sb[:, kc, :], d_wi1[kc * 128:(kc + 1) * 128, :])
            b1_sb = cpool.tile([1, G4], f32r, tag="brow0")
            nc.sync.dma_start(b1_sb[:, :], d_b1[:, :])

            wh_sb = wpool.tile([128, 4, G4], f32r, tag="wh")  # Wh1, later Wh2
            for kc in range(4):
                nc.sync.dma_start(wh_sb[:, kc, :], d_wh1[kc * 128:(kc + 1) * 128, :])

            # ---- DRAM intermediates ----
            z1d = dpool.tile([NCH, 128, G4], bf16, tag="z1d")
            z2d = dpool.tile([NCH, 128, G4], bf16, tag="z2d")
            h1t = dpool.tile([128, 4, t_steps, 16], f32r, tag="h1t")
            cc1_in = dpool.tile([128, 8], f32, tag="cc1i")
            cc1_out = dpool.tile([128, 8], f32, tag="cc1o")
            cc2_in = dpool.tile([128, 8], f32, tag="cc2i")
            cc2_out = dpool.tile([128, 8], f32, tag="cc2o")

            # ================= Phase A: Z1 precompute =================
            def z_precompute(zd, get_lhs_tile, rhs_w, bias_row):
                """zd[c] = lhsT_c.T @ W + bias_row for all row chunks (bf16)."""
                for c in range(NCH):
                    lhs = [get_lhs_tile(c, kc) for kc in range(4)]
                    zp = pspool.tile([128, G4], f32, tag="ps", name="zp")
                    for kc in range(4):
                        for nb in range(4):
                            nc.tensor.matmul(
                                zp[:, nb * 512:(nb + 1) * 512],
                                lhs[kc][:, :],
                                rhs_w[:, kc, nb * 512:(nb + 1) * 512],
                                start=(kc == 0), stop=False)
                    for nb in range(4):
                        nc.tensor.matmul(
                            zp[:, nb * 512:(nb + 1) * 512],
                            ones_sb[:, :],
                            bias_row[:, nb * 512:(nb + 1) * 512],
                            start=False, stop=True)
                    zsb = zpool.tile([128, G4], bf16, tag="zstr", name="zsb")
                    nc.vector.tensor_copy(zsb[:, :], zp[:, :])
                    nc.sync.dma_start(zd[c], zsb[:, :])

            def get_x_tile(c, kc):
                xt = xpool.tile([128, 128], f32r, tag="xt", name="xt")
                nc.sync.dma_start(
                    xt[:, :], d_xT[kc * 128:(kc + 1) * 128, c * 128:(c + 1) * 128])
                return xt

            z_precompute(z1d, get_x_tile, wi_sb, b1_sb)

            # ---- persistent recurrence state ----
            hT_sb = cpool.tile([128, 4, 16], f32r, tag="hT")
            c_sb = cpool.tile([16, 512], bf16, tag="cst")

            # ================= recurrence =================
            def lstm_layer(zd, wh, store_h1t):
                nc.vector.memset(c_sb[:, :], 0.0)
                zch = {0: zpool.tile([128, G4], bf16, tag="zstr", name="zch0")}
                nc.sync.dma_start(zch[0][:, :], zd[0])
                for t in range(t_steps):
                    st0 = (t == 0)  # h == 0: skip the Wh matmuls
                    cix, ts = divmod(t, SPC)
                    if ts == 0 and cix + 1 < NCH:
                        zch[cix + 1] = zpool.tile([128, G4], bf16, tag="zstr",
                                                  name="zch")
                        nc.sync.dma_start(zch[cix + 1][:, :], zd[cix + 1])
                    if cix - 2 in zch:
                        del zch[cix - 2]
                    base = 32 * (ts // 2)
                    sel = ia_bf if ts % 2 == 0 else ib_bf
                    zp = pspool.tile([16, G4], f32, tag="ps", name="rzp")
                    gates = gpool.tile([16, G4], bf16, tag="gates", name="gates")
                    t1 = tpool.tile([16, 512], bf16, tag="t1", name="t1")
                    t2 = tpool.tile([16, 512], bf16, tag="t2", name="t2")
                    tcs = tpool.tile([16, 512], bf16, tag="tc", name="tcs")
                    hs = tpool.tile([16, 512], f32, tag="h", name="hs")

                    # nb-outer: each gate block's accumulation stops as soon
                    # as its 5 matmuls are done, so the ACT/DVE chain overlaps
                    # the remaining blocks. Block order g(3), i(0), f(1), o(2)
                    # leaves only sigmoid(o) -> h after the last matmul.
                    # gates/c in bf16 (DVE 2x); PSUM accum and h stay fp32.
                    def block(nb):
                        cs = slice(nb * 512, (nb + 1) * 512)
                        nc.tensor.matmul(
                            zp[:, cs], sel[base:base + 32, :],
                            zch[cix][base:base + 32, cs],
                            start=True, stop=st0, tile_position=(base, 0))
                        if not st0:
                            for kc in range(4):
                                nc.tensor.matmul(
                                    zp[:, cs], hT_sb[:, kc, :],
                                    wh[:, kc, cs],
                                    start=False, stop=(kc == 3))

                    block(3)
                    block(0)
                    nc.scalar.activation(gates[:, 1536:G4], zp[:, 1536:G4],
                                         AF.Tanh)
                    block(1)
                    nc.scalar.activation(gates[:, 0:1024], zp[:, 0:1024],
                                         AF.Sigmoid)
                    nc.vector.tensor_mul(t1[:, :], gates[:, 0:512],
                                         gates[:, 1536:G4])
                    nc.vector.tensor_mul(t2[:, :], gates[:, 512:1024], c_sb[:, :])
                    block(2)
                    nc.vector.tensor_add(c_sb[:, :], t1[:, :], t2[:, :])
                    nc.scalar.activation(tcs[:, :], c_sb[:, :], AF.Tanh)
                    nc.scalar.activation(gates[:, 1024:1536], zp[:, 1024:1536],
                                         AF.Sigmoid)
                    nc.vector.tensor_mul(hs[:, :], gates[:, 1024:1536], tcs[:, :])
                    htp = pspool.tile([128, 4, 16], f32, tag="ps", name="htp")
                    for kc in range(4):
                        nc.tensor.matmul(
                            htp[:, kc, :], hs[:, kc * 128:(kc + 1) * 128],
                            i16_sb[:, :], start=(kc == 0), stop=(kc == 3),
                            is_transpose=True)
                    nc.vector.tensor_copy(hT_sb[:, :, :], htp[:, :, :])
                    if store_h1t:
                        nc.sync.dma_start(h1t[:, :, t, :], hT_sb[:, :, :])

            lstm_layer(z1d, wh_sb, store_h1t=True)

            # ================= Phase C: BN1 stats =================
            psum_parts = cpool.tile([128, 4, 4], f32, tag="p_sum")
            psq_parts = cpool.tile([128, 4, 4], f32, tag="p_sq")
            TCH = t_steps // 4  # stat chunk in timesteps
            for qi in reversed(range(4)):
                for kc in range(4):
                    hb = zpool.tile([128, TCH, 16], f32r, tag="zstr", name="hb")
                    nc.sync.dma_start(
                        hb[:, :, :], h1t[:, kc, qi * TCH:(qi + 1) * TCH, :])
                    hbv = hb[:, :, :].bitcast(f32)
                    tr1 = tpool.tile([128, TCH, 16], bf16, tag="trash", name="tr1")
                    nc.scalar.activation(tr1[:, :, :], hbv, AF.Identity,
                                         accum_out=psum_parts[:, kc, qi:qi + 1])
                    tr2 = tpool.tile([128, TCH, 16], bf16, tag="trash", name="tr2")
                    nc.scalar.activation(tr2[:, :, :], hbv, AF.Square,
                                         accum_out=psq_parts[:, kc, qi:qi + 1])
            allred = cpool.tile([128, 8], f32, tag="allred")
            nc.vector.tensor_reduce(allred[:, 0:4], psum_parts[:, :, :],
                                    mybir.AxisListType.X, mybir.AluOpType.add)
            nc.vector.tensor_reduce(allred[:, 4:8], psq_parts[:, :, :],
                                    mybir.AxisListType.X, mybir.AluOpType.add)
            nc.sync.dma_start(cc1_in[:, :], allred[:, :])
            nc.gpsimd.collective_compute(
                "AllReduce", mybir.AluOpType.add,
                replica_groups=[list(range(NCORES))],
                ins=[cc1_in.opt()], outs=[cc1_out.opt()])
            nc.sync.dma_start(allred[:, :], cc1_out[:, :])

            bn1s_sb = cpool.tile([128, 4], f32, tag="bn1s")
            bn1b_sb = cpool.tile([128, 4], f32, tag="bn1b")
            nc.sync.dma_start(bn1s_sb[:, :], d_bn1s[:, :])
            nc.sync.dma_start(bn1b_sb[:, :], d_bn1b[:, :])

            def bn_fold(allred_sb, n_count, bns, bnb):
                """Return (a, d): bn(x) = x*a + d per feature, [128,4] tiles."""
                mu = cpool.tile([128, 4], f32, tag=f"mu{n_count}")
                ex2 = cpool.tile([128, 4], f32, tag=f"ex2{n_count}")
                nc.vector.tensor_scalar_mul(mu[:, :], allred_sb[:, 0:4], 1.0 / n_count)
                nc.vector.tensor_scalar_mul(ex2[:, :], allred_sb[:, 4:8], 1.0 / n_count)
                var = cpool.tile([128, 4], f32, tag=f"var{n_count}")
                nc.vector.tensor_mul(var[:, :], mu[:, :], mu[:, :])
                nc.vector.tensor_sub(var[:, :], ex2[:, :], var[:, :])
                nc.vector.tensor_scalar_add(var[:, :], var[:, :], EPS)
                sd = cpool.tile([128, 4], f32, tag=f"sd{n_count}")
                nc.scalar.activation(sd[:, :], var[:, :], AF.Sqrt)
                r0 = cpool.tile([128, 4], f32, tag=f"r0{n_count}")
                nc.vector.reciprocal(r0[:, :], sd[:, :])
                # one Newton step: r1 = r0 * (1.5 - 0.5 * var * r0^2)
                e1 = cpool.tile([128, 4], f32, tag=f"e1{n_count}")
                nc.vector.tensor_mul(e1[:, :], r0[:, :], r0[:, :])
                nc.vector.tensor_mul(e1[:, :], e1[:, :], var[:, :])
                nc.vector.tensor_scalar(e1[:, :], e1[:, :], -0.5, 1.5,
                                        mybir.AluOpType.mult, mybir.AluOpType.add)
                nc.vector.tensor_mul(r0[:, :], r0[:, :], e1[:, :])
                a = cpool.tile([128, 4], f32, tag=f"a{n_count}")
                dv = cpool.tile([128, 4], f32, tag=f"d{n_count}")
                nc.vector.tensor_mul(a[:, :], r0[:, :], bns[:, :])
                nc.vector.tensor_mul(dv[:, :], mu[:, :], a[:, :])
                nc.vector.tensor_sub(dv[:, :], bnb[:, :], dv[:, :])
                return a, dv

            a1, d1v = bn_fold(allred, B * t_steps, bn1s_sb, bn1b_sb)

            # ================= Phase D: fold BN1 into Wi2 =================
            for kc in range(4):
                nc.sync.dma_start(wi_sb[:, kc, :], d_wi2[kc * 128:(kc + 1) * 128, :])
            b2_sb = cpool.tile([1, G4], f32r, tag="brow1")
            nc.sync.dma_start(b2_sb[:, :], d_b2[:, :])

            dvr = cpool.tile([128, 4], f32r, tag="dvr")
            nc.vector.tensor_copy(dvr[:, :], d1v[:, :])
            r2_ps = pspool.tile([1, G4], f32, tag="ps", name="r2ps")
            for kc in range(4):
                for nb in range(4):
                    nc.tensor.matmul(r2_ps[:, nb * 512:(nb + 1) * 512],
                                     dvr[:, kc:kc + 1],
                                     wi_sb[:, kc, nb * 512:(nb + 1) * 512],
                                     start=(kc == 0), stop=False)
            for nb in range(4):
                nc.tensor.matmul(r2_ps[:, nb * 512:(nb + 1) * 512],
                                 ones_sb[:, 0:1], b2_sb[:, nb * 512:(nb + 1) * 512],
                                 start=False, stop=True)
            r2_sb = cpool.tile([1, G4], f32r, tag="brow0r")
            nc.vector.tensor_copy(r2_sb[:, :], r2_ps[:, :])
            for kc in range(4):
                nc.vector.tensor_scalar_mul(wi_sb[:, kc, :], wi_sb[:, kc, :],
                                            a1[:, kc:kc + 1])

            # ================= Phase E: Z2 precompute =================
            def get_h1t_tile(c, kc):
                ht = xpool.tile([128, SPC, 16], f32r, tag="xt", name="htl")
                nc.sync.dma_start(ht[:, :, :],
                                  h1t[:, kc, c * SPC:(c + 1) * SPC, :])
                return ht

            z_precompute(z2d, get_h1t_tile, wi_sb, r2_sb)

            # ================= Phase F: L2 recurrence =================
            for kc in range(4):
                nc.sync.dma_start(wh_sb[:, kc, :], d_wh2[kc * 128:(kc + 1) * 128, :])
            lstm_layer(z2d, wh_sb, store_h1t=False)

            # ================= Phase G: BN2 + dense head =================
            hT32 = cpool.tile([128, 4, 16], f32, tag="hT32")
            nc.vector.tensor_copy(hT32[:, :, :], hT_sb[:, :, :])
            s2 = cpool.tile([128, 4], f32, tag="s2")
            q2 = cpool.tile([128, 4], f32, tag="q2")
            tr3 = cpool.tile([128, 4, 16], bf16, tag="tr3")
            for kc in range(4):
                nc.scalar.activation(tr3[:, kc, :], hT32[:, kc, :], AF.Identity,
                                     accum_out=s2[:, kc:kc + 1])
                nc.scalar.activation(tr3[:, kc, :], hT32[:, kc, :], AF.Square,
                                     accum_out=q2[:, kc:kc + 1])
            allred2 = cpool.tile([128, 8], f32, tag="allred2")
            nc.vector.tensor_copy(allred2[:, 0:4], s2[:, :])
            nc.vector.tensor_copy(allred2[:, 4:8], q2[:, :])
            nc.sync.dma_start(cc2_in[:, :], allred2[:, :])
            nc.gpsimd.collective_compute(
                "AllReduce", mybir.AluOpType.add,
                replica_groups=[list(range(NCORES))],
                ins=[cc2_in.opt()], outs=[cc2_out.opt()])
            nc.sync.dma_start(allred2[:, :], cc2_out[:, :])

            bn2s_sb = cpool.tile([128, 4], f32, tag="bn2s")
            bn2b_sb = cpool.tile([128, 4], f32, tag="bn2b")
            nc.sync.dma_start(bn2s_sb[:, :], d_bn2s[:, :])
            nc.sync.dma_start(bn2b_sb[:, :], d_bn2b[:, :])
            a2, d2v = bn_fold(allred2, B, bn2s_sb, bn2b_sb)

            wd1_sb = cpool.tile([128, 4, 16], f32, tag="wd1")
            for kc in range(4):
                nc.sync.dma_start(wd1_sb[:, kc, :], d_wd1[kc * 128:(kc + 1) * 128, :])
            bd1_sb = cpool.tile([16, 1], f32, tag="bd1")
            nc.sync.dma_start(bd1_sb[:, :], d_bd1[:, :])
            wd2_sb = cpool.tile([16, 1], f32, tag="wd2")
            nc.sync.dma_start(wd2_sb[:, :], d_wd2[:, :])
            bd2_sb = cpool.tile([1, 1], f32, tag="bd2")
            nc.sync.dma_start(bd2_sb[:, :], d_bd2[:, :])

            # bias_d1[j] = sum_h Wd1[h, j] * d2v[h] + bd1[j]  (psum [16, 1])
            bd1_ps = pspool.tile([16, 1], f32, tag="ps", name="bd1ps")
            for kc in range(4):
                nc.tensor.matmul(bd1_ps[:, :], wd1_sb[:, kc, :], d2v[:, kc:kc + 1],
                                 start=(kc == 0), stop=(kc == 3))
            biasd1 = cpool.tile([16, 1], f32, tag="biasd1")
            nc.vector.tensor_copy(biasd1[:, :], bd1_ps[:, :])
            nc.vector.tensor_add(biasd1[:, :], biasd1[:, :], bd1_sb[:, :])
            # scale Wd1 rows by a2 (after the bias matmuls read the raw Wd1)
            for kc in range(4):
                nc.vector.tensor_scalar_mul(wd1_sb[:, kc, :], wd1_sb[:, kc, :],
                                            a2[:, kc:kc + 1])
            # d1T[j, b] = tanh( sum_h Wd1'[h,j] * hT[h,b] + bias_d1[j] )
            d1_ps = pspool.tile([16, 16], f32, tag="ps", name="d1ps")
            for kc in range(4):
                nc.tensor.matmul(d1_ps[:, :], wd1_sb[:, kc, :], hT32[:, kc, :],
                                 start=(kc == 0), stop=(kc == 3))
            d1T = cpool.tile([16, 16], f32, tag="d1T")
            nc.scalar.activation(d1T[:, :], d1_ps[:, :], AF.Tanh, bias=biasd1[:, 0:1])
            # out[0, b] = sum_j Wd2[j] * d1T[j, b] + bd2
            o_ps = pspool.tile([1, 16], f32, tag="ps", name="ops")
            nc.tensor.matmul(o_ps[:, :], wd2_sb[:, :], d1T[:, :],
                             start=True, stop=True)
            out_sb = cpool.tile([1, 16], f32, tag="outsb")
            nc.scalar.activation(out_sb[:, :], o_ps[:, :], AF.Identity,
                                 bias=bd2_sb[:, 0:1])
            nc.sync.dma_start(d_out[:, :], out_sb[:, :])

    nc.compile()
    return nc


_PROG_CACHE = {}


def _get_program(t_steps):
    if t_steps not in _PROG_CACHE:
        _PROG_CACHE[t_steps] = _build_program(t_steps)
    return _PROG_CACHE[t_steps]


def kernel(x, Wi1, Wh1, b1, Wi2, Wh2, b2, bn1_scale, bn1_bias,
           bn2_scale, bn2_bias, Wd1, bd1, Wd2, bd2):
    from concourse.bass_utils import run_bass_kernel_spmd

    x = np.asarray(x, dtype=np.float32)
    t_steps = x.shape[1]
    nc = _get_program(t_steps)

    # gate reorder (i,f,g,o) -> (i,f,o,g)
    perm = np.concatenate([np.arange(0, 512), np.arange(512, 1024),
                           np.arange(1536, 2048), np.arange(1024, 1536)])
    wi1 = np.ascontiguousarray(np.asarray(Wi1, np.float32)[:, perm])
    wh1 = np.ascontiguousarray(np.asarray(Wh1, np.float32)[:, perm])
    b1p = np.asarray(b1, np.float32)[perm].reshape(1, G4)
    wi2 = np.ascontiguousarray(np.asarray(Wi2, np.float32)[:, perm])
    wh2 = np.ascontiguousarray(np.asarray(Wh2, np.float32)[:, perm])
    b2p = np.asarray(b2, np.float32)[perm].reshape(1, G4)

    def col4(v):
        return np.ascontiguousarray(np.asarray(v, np.float32).reshape(4, 128).T)

    ia = np.zeros((128, 16), np.float32)
    ib = np.zeros((128, 16), np.float32)
    for g in range(4):
        for j in range(16):
            ia[32 * g + j, j] = 1.0
            ib[32 * g + 16 + j, j] = 1.0
    common = {
        "wi1": wi1, "wh1": wh1, "b1row": b1p,
        "wi2": wi2, "wh2": wh2, "b2row": b2p,
        "bn1s": col4(bn1_scale), "bn1b": col4(bn1_bias),
        "bn2s": col4(bn2_scale), "bn2b": col4(bn2_bias),
        "wd1": np.asarray(Wd1, np.float32),
        "bd1c": np.asarray(bd1, np.float32).reshape(16, 1),
        "wd2": np.asarray(Wd2, np.float32).reshape(16, 1),
        "bd2c": np.asarray(bd2, np.float32).reshape(1, 1),
        "IA": ia, "IB": ib, "I16": np.eye(16, dtype=np.float32),
        "ones1": np.ones((1, 128), np.float32),
    }
    in_maps = []
    for ci in range(NCORES):
        xs = x[ci * BL:(ci + 1) * BL]                    # [16, T, F]
        xT = np.ascontiguousarray(xs.transpose(2, 1, 0).reshape(F, t_steps * BL))
        m = dict(common)
        m["xT"] = xT
        in_maps.append(m)

    global _LAST_IN_MAPS
    _LAST_IN_MAPS = in_maps
    res = run_bass_kernel_spmd(nc, in_maps, core_ids=list(range(NCORES)))
    y = np.concatenate(
        [res.results[ci]["out"].reshape(16, 1) for ci in range(NCORES)], axis=0)
    return y.astype(np.float32)
